# revision 12
# baseline (speedup 1.0000x reference)
"""DGCNN-cls Trainium2 Bass kernel (8 NeuronCores, data-parallel batch+points).

Sharding: core c handles batch b=c//2, point rows [(c%2)*1024, (c%2)*1024+1024).
Pair collectives (AllGather features / AllReduce-max global pool) between cores
2b and 2b+1. kernel() accepts FULL inputs, returns FULL outputs.
"""

import numpy as np

import concourse.bass as bass
import concourse.bacc as bacc
import concourse.tile as tile
from concourse import mybir
from concourse.bass_utils import run_bass_kernel_spmd
from concourse.masks import make_identity

F32 = mybir.dt.float32
F32R = mybir.dt.float32r
U16 = mybir.dt.uint16
I16 = mybir.dt.int16

B = 4
C0 = 6
N = 2048
K = 20
HALF = 1024
NCHUNK = 8
P = 128
SEG = 64
NSEG = N // SEG
EPS = 1e-5
NEG = -1e30

AF = mybir.ActivationFunctionType
ALU = mybir.AluOpType


def _sap(ap, pairs, extra_off=0):
    """new AP on same tensor with custom [step,count] pairs."""
    return bass.AP(tensor=ap.tensor, offset=ap.offset + extra_off, ap=pairs)


def build_nc(mm_dt=F32, score_dt=F32):
    nc = bacc.Bacc("TRN2", target_bir_lowering=False, debug=False, num_devices=8)

    def inp(name, shape):
        return nc.dram_tensor(name, shape, F32, kind="ExternalInput")

    x_full = inp("x_full", [C0, N])
    x_own = inp("x_own", [C0, HALF])
    w1t = inp("w1t", [C0, 64]); w1b = inp("w1b", [C0, 64])
    w1l2 = inp("w1l2", [64, 64])
    s1 = inp("s1", [64, 1]); b1 = inp("b1", [64, 1])
    s11 = inp("s11", [64, 1]); b11 = inp("b11", [64, 1])
    w2t = inp("w2t", [64, 64]); w2b = inp("w2b", [64, 64])
    w2l2 = inp("w2l2", [64, 64])
    s2 = inp("s2", [64, 1]); b2 = inp("b2", [64, 1])
    s22 = inp("s22", [64, 1]); b22 = inp("b22", [64, 1])
    w3t = inp("w3t", [64, 64]); w3b = inp("w3b", [64, 64])
    s3 = inp("s3", [64, 1]); b3 = inp("b3", [64, 1])
    w5w = inp("w5w", [192, 1024])
    s5 = inp("s5", [P, 8]); b5 = inp("b5", [P, 8])
    w6g = inp("w6g", [1024, 512])
    w6c = inp("w6c", [192, 512])
    s6 = inp("s6", [P, 4]); b6 = inp("b6", [P, 4])
    w7w = inp("w7w", [512, 256])
    s7 = inp("s7", [P, 2]); b7 = inp("b7", [P, 2])
    w8w = inp("w8w", [256, 128])
    w9w = inp("w9w", [128, 1])
    w11w = inp("w11w", [256, 128])

    o_pdist = nc.dram_tensor("o_pdist", [HALF, N], F32, kind="ExternalOutput")
    o_simmat = nc.dram_tensor("o_simmat", [HALF, N], F32, kind="ExternalOutput")
    o_fsim = nc.dram_tensor("o_fsim", [HALF, 128], F32, kind="ExternalOutput")
    o_score = nc.dram_tensor("o_score", [1, HALF], F32, kind="ExternalOutput")

    idx_scr = nc.dram_tensor("idx_scr", [NCHUNK, P, 24], U16, kind="Internal")
    cc_in_x1 = nc.dram_tensor("cc_in_x1", [64, HALF], F32, kind="Internal")
    cc_out_x1 = nc.dram_tensor("cc_out_x1", [2, 64, HALF], F32, kind="Internal")
    cc_in_x2 = nc.dram_tensor("cc_in_x2", [64, HALF], F32, kind="Internal")
    cc_out_x2 = nc.dram_tensor("cc_out_x2", [2, 64, HALF], F32, kind="Internal")
    cc_in_g = nc.dram_tensor("cc_in_g", [P, 8], F32, kind="Internal")
    cc_out_g = nc.dram_tensor("cc_out_g", [P, 8], F32, kind="Internal")
    cc_in_fs = nc.dram_tensor("cc_in_fs", [129, HALF], F32, kind="Internal")
    cc_out_fs = nc.dram_tensor("cc_out_fs", [2, 129, HALF], F32, kind="Internal")
    GROUPS = [[0, 1], [2, 3], [4, 5], [6, 7]]

    with tile.TileContext(nc) as tc:
        with tc.tile_pool(name="persist", bufs=1) as pp:
            # ---------- weights ----------
            _ldn = [0]

            def load(t):
                _ldn[0] += 1
                tl = pp.tile(list(t.shape), F32, tag=f"ld{_ldn[0]}",
                             name=f"ld{_ldn[0]}")
                nc.sync.dma_start(out=tl[:, :], in_=t[:, :])
                return tl

            W1t, W1b, W1l2 = load(w1t), load(w1b), load(w1l2)
            S1, B1, S11, B11 = load(s1), load(b1), load(s11), load(b11)
            W2t, W2b, W2l2 = load(w2t), load(w2b), load(w2l2)
            S2, B2, S22, B22 = load(s2), load(b2), load(s22), load(b22)
            W3t, W3b = load(w3t), load(w3b)
            S3, B3 = load(s3), load(b3)
            W5a = pp.tile([128, 1024], F32)
            nc.sync.dma_start(out=W5a[:, :], in_=w5w[0:128, :])
            W5b = pp.tile([64, 1024], F32)
            nc.sync.dma_start(out=W5b[:, :], in_=w5w[128:192, :])
            S5, B5 = load(s5), load(b5)
            W6g = []
            for kk in range(8):
                t = pp.tile([128, 512], F32, tag=f"w6g{kk}", name=f"w6gt{kk}")
                nc.sync.dma_start(out=t[:, :], in_=w6g[kk * 128:(kk + 1) * 128, :])
                W6g.append(t)
            W6ca = pp.tile([128, 512], F32)
            nc.sync.dma_start(out=W6ca[:, :], in_=w6c[0:128, :])
            W6cb = pp.tile([64, 512], F32)
            nc.sync.dma_start(out=W6cb[:, :], in_=w6c[128:192, :])
            S6, B6 = load(s6), load(b6)
            W7 = []
            for kk in range(4):
                t = pp.tile([128, 256], F32, tag=f"w7_{kk}", name=f"w7t{kk}")
                nc.sync.dma_start(out=t[:, :], in_=w7w[kk * 128:(kk + 1) * 128, :])
                W7.append(t)
            S7, B7 = load(s7), load(b7)
            W8 = []
            for kk in range(2):
                t = pp.tile([128, 128], F32, tag=f"w8_{kk}", name=f"w8t{kk}")
                nc.sync.dma_start(out=t[:, :], in_=w8w[kk * 128:(kk + 1) * 128, :])
                W8.append(t)
            W9 = load(w9w)
            W11 = []
            for kk in range(2):
                t = pp.tile([128, 128], F32, tag=f"w11_{kk}", name=f"w11t{kk}")
                nc.sync.dma_start(out=t[:, :], in_=w11w[kk * 128:(kk + 1) * 128, :])
                W11.append(t)

            # ---------- features ----------
            f_ext = pp.tile([16, N], F32)
            nc.vector.memset(f_ext[:, :], 0.0)
            nc.sync.dma_start(out=f_ext[0:C0, :], in_=x_full[:, :])
            x_own_sb = pp.tile([C0, HALF], F32)
            nc.sync.dma_start(out=x_own_sb[:, :], in_=x_own[:, :])

            x1T = pp.tile([66, N], F32)
            x2T = pp.tile([66, N], F32)
            x1own = pp.tile([64, HALF], F32)
            x2own = pp.tile([64, HALF], F32)
            cat_a = pp.tile([P, HALF], F32)
            cat_b = pp.tile([64, HALF], F32)
            ones_col = pp.tile([P, 1], F32)
            nc.vector.memset(ones_col[:, :], 1.0)
            ones_row = pp.tile([1, P], F32)
            nc.vector.memset(ones_row[:, :], 1.0)
            ones_row_n = pp.tile([1, N], F32)
            nc.vector.memset(ones_row_n[:, :], 1.0)

            # ============ stages ============
            with (
                tc.tile_pool(name="stage", bufs=1) as sp,
                tc.tile_pool(name="sbuf2", bufs=2) as sp2,
                tc.tile_pool(name="spsum", bufs=1, space="PSUM") as spsum,
                tc.tile_pool(name="epsum", bufs=2, space="PSUM") as epsum,
            ):
                def colsq_row(dst_row_ap, src, nch, width, tagn):
                    """dst_row_ap [1,width] (any partition) = -sum_c src**2."""
                    sq = sp2.tile([64, width], F32, tag="sqtmp")
                    nc.scalar.activation(sq[0:nch, :], src, AF.Square)
                    ps = spsum.tile([1, width], F32, tag="spsum")
                    for j in range(0, width, 512):
                        nc.tensor.matmul(ps[:, j:j + 512], ones_col[0:nch, :],
                                         sq[0:nch, j:j + 512], start=True,
                                         stop=True)
                    row = sp.tile([1, width], F32, tag=tagn, name=tagn)
                    nc.scalar.activation(row[:, :], ps[:, :], AF.Copy,
                                         bias=0.0, scale=-1.0)
                    nc.sync.dma_start(out=dst_row_ap, in_=row[:, :])

                def run_stage(stage, feat_full, featC, qsrc, out_dst, out_p0):
                    CC = featC
                    KQ = CC + 2
                    if stage == 1:
                        Wt, Wb, Wl2 = W1t, W1b, W1l2
                        Sa, Ba, Sb, Bb = S1, B1, S11, B11
                    elif stage == 2:
                        Wt, Wb, Wl2 = W2t, W2b, W2l2
                        Sa, Ba, Sb, Bb = S2, B2, S22, B22
                    else:
                        Wt, Wb, Wl2 = W3t, W3b, None
                        Sa, Ba, Sb, Bb = S3, B3, None, None

                    colsq_row(feat_full[CC + 1:CC + 2, :], feat_full[0:CC, :],
                              CC, N, "nsqf")
                    nc.sync.dma_start(out=feat_full[CC:CC + 1, :],
                                      in_=ones_row_n[:, :])

                    q = sp.tile([66, HALF], F32, tag="qtile")
                    nc.scalar.activation(q[0:CC, :], qsrc, AF.Copy, bias=0.0,
                                         scale=2.0)
                    colsq_row(q[CC:CC + 1, :], qsrc, CC, HALF, "nsqq")
                    nc.sync.dma_start(out=q[CC + 1:CC + 2, :],
                                      in_=ones_row_n[:, 0:HALF])

                    idxw = sp.tile([64, NCHUNK * 160], I16, tag="idxw")

                    for m in range(NCHUNK):
                        ps = spsum.tile([P, N], F32, tag="spsum")
                        qs = q[0:KQ, m * P:(m + 1) * P]
                        for j in range(4):
                            nc.tensor.matmul(
                                ps[:, j * 512:(j + 1) * 512],
                                qs.bitcast(score_dt),
                                feat_full[0:KQ, j * 512:(j + 1) * 512]
                                .bitcast(score_dt),
                                start=True, stop=True)
                        s_sb = sp2.tile([P, N], F32, tag="s_sb")
                        nc.scalar.activation(s_sb[:, :], ps[:, :], AF.Copy,
                                             bias=0.0, scale=1.0)
                        if stage == 1:
                            d2 = sp2.tile([P, N], F32, tag="d2")
                            nc.scalar.activation(d2[:, :], ps[:, :], AF.Copy,
                                                 bias=0.0, scale=-1.0)
                            nc.sync.dma_start(
                                out=o_pdist[m * P:(m + 1) * P, :], in_=d2[:, :])

                        cands = sp.tile([P, 256], F32, tag="cands")
                        for g in range(NSEG):
                            nc.vector.max(cands[:, g * 8:(g + 1) * 8],
                                          s_sb[:, g * SEG:(g + 1) * SEG])
                        c1 = sp.tile([P, 256], F32, tag="c1")
                        c2 = sp.tile([P, 256], F32, tag="c2")
                        v8 = [sp.tile([P, 8], F32, tag=f"v8_{r}", name=f"v8t_{r}")
                              for r in range(3)]
                        nc.vector.max(v8[0][:, :], cands[:, :])
                        nc.vector.match_replace(c1[:, :], v8[0][:, :],
                                                cands[:, :], NEG)
                        nc.vector.max(v8[1][:, :], c1[:, :])
                        nc.vector.match_replace(c2[:, :], v8[1][:, :],
                                                c1[:, :], NEG)
                        nc.vector.max(v8[2][:, :], c2[:, :])
                        idx24 = sp.tile([P, 24], U16, tag="idx24")
                        for r in range(3):
                            nc.vector.max_index(idx24[:, r * 8:(r + 1) * 8],
                                                v8[r][:, :], s_sb[:, :])
                        nc.sync.dma_start(out=idx_scr[m, :, :], in_=idx24[:, :])

                    # readback wrapped idx: element j=k*128+p at [p%16, k*8+p//16]
                    ngrp = 4 if CC >= 16 else 1
                    for m in range(NCHUNK):
                        src = bass.AP(idx_scr, m * P * 24,
                                      [[24, 16], [16 * 24, 8], [1, 20]])
                        for g in range(ngrp):
                            nc.sync.dma_start(
                                out=idxw[g * 16:(g + 1) * 16,
                                         m * 160:(m + 1) * 160],
                                in_=src.bitcast(I16))

                    for m in range(NCHUNK):
                        nb = sp2.tile([64, K * P], F32, tag="nb")
                        if CC >= 16:
                            nc.gpsimd.ap_gather(
                                out_ap=nb[:, :], in_ap=feat_full[0:64, :],
                                idxs_ap=idxw[:, m * 160:(m + 1) * 160],
                                channels=64, num_elems=N, d=1, num_idxs=K * P)
                        else:
                            nc.gpsimd.ap_gather(
                                out_ap=nb[0:16, :], in_ap=f_ext[0:16, :],
                                idxs_ap=idxw[0:16, m * 160:(m + 1) * 160],
                                channels=16, num_elems=N, d=1, num_idxs=K * P)
                        h2 = sp2.tile([64, K * P], F32, tag="h2")
                        for pc in range(8):
                            sl = slice(pc * 320, (pc + 1) * 320)
                            c16 = qsrc[:, m * P + pc * 16:m * P + pc * 16 + 16]
                            ctr_b = _sap(c16, [c16.ap[0], [0, K], c16.ap[1]])
                            e1 = epsum.tile([64, 320], F32, tag="e1")
                            nc.tensor.matmul(e1[:, :], Wt[:, :].bitcast(mm_dt),
                                             nb[0:CC, sl].bitcast(mm_dt),
                                             start=True, stop=False)
                            nc.tensor.matmul(e1[:, :], Wb[:, :].bitcast(mm_dt),
                                             ctr_b.bitcast(mm_dt),
                                             start=False, stop=True)
                            if Wl2 is not None:
                                h1 = sp.tile([64, 320], F32, tag="h1")
                                nc.scalar.activation(h1[:, :], e1[:, :],
                                                     AF.Relu, bias=Ba[:, :],
                                                     scale=Sa[:, :])
                                e2 = epsum.tile([64, 320], F32, tag="e2")
                                nc.tensor.matmul(e2[:, :],
                                                 Wl2[:, :].bitcast(mm_dt),
                                                 h1[:, :].bitcast(mm_dt),
                                                 start=True, stop=True)
                                nc.scalar.activation(h2[:, sl], e2[:, :],
                                                     AF.Relu, bias=Bb[:, :],
                                                     scale=Sb[:, :])
                            else:
                                nc.scalar.activation(h2[:, sl], e1[:, :],
                                                     AF.Relu, bias=Ba[:, :],
                                                     scale=Sa[:, :])
                        h2ap = h2[:, :]
                        red_in = _sap(h2ap, [h2ap.ap[0], [320, 8], [1, 16],
                                             [16, K]])
                        nc.vector.reduce_max(
                            out_dst[out_p0:out_p0 + 64, m * P:(m + 1) * P],
                            red_in, axis=mybir.AxisListType.X)

                run_stage(1, f_ext, C0, x_own_sb[:, :], x1own, 0)

                nc.sync.dma_start(out=cc_in_x1[:, :], in_=x1own[:, :])
                nc.sync.dma_start(out=cat_a[0:64, :], in_=x1own[:, :])
                nc.gpsimd.collective_compute(
                    "AllGather", ALU.bypass, replica_groups=GROUPS,
                    ins=[cc_in_x1.ap()], outs=[cc_out_x1.ap()])
                for h in range(2):
                    nc.sync.dma_start(out=x1T[0:64, h * HALF:(h + 1) * HALF],
                                      in_=cc_out_x1[h])

                run_stage(2, x1T, 64, x1own[:, :], x2own, 0)

                nc.sync.dma_start(out=cc_in_x2[:, :], in_=x2own[:, :])
                nc.sync.dma_start(out=cat_a[64:128, :], in_=x2own[:, :])
                nc.gpsimd.collective_compute(
                    "AllGather", ALU.bypass, replica_groups=GROUPS,
                    ins=[cc_in_x2.ap()], outs=[cc_out_x2.ap()])
                for h in range(2):
                    nc.sync.dma_start(out=x2T[0:64, h * HALF:(h + 1) * HALF],
                                      in_=cc_out_x2[h])

                run_stage(3, x2T, 64, x2own[:, :], cat_b, 0)

            # ============ head ============
            with (
                tc.tile_pool(name="persist2", bufs=1) as pp2,
                tc.tile_pool(name="h2buf", bufs=2) as hb2,
                tc.tile_pool(name="hpsum", bufs=2, space="PSUM") as hpsum,
                tc.tile_pool(name="hpsmall", bufs=2, space="PSUM") as hpsmall,
            ):
                pg = pp2.tile([P, 8], F32)
                for mt in range(8):
                    ps = hpsum.tile([P, HALF], F32, tag="hps")
                    for j in range(2):
                        sl = slice(j * 512, (j + 1) * 512)
                        nc.tensor.matmul(
                            ps[:, sl],
                            W5a[:, mt * 128:(mt + 1) * 128].bitcast(mm_dt),
                            cat_a[:, sl].bitcast(mm_dt), start=True, stop=False)
                        nc.tensor.matmul(
                            ps[:, sl],
                            W5b[:, mt * 128:(mt + 1) * 128].bitcast(mm_dt),
                            cat_b[:, sl].bitcast(mm_dt), start=False, stop=True)
                    h5 = hb2.tile([P, HALF], F32, tag="h5")
                    nc.scalar.activation(h5[:, :], ps[:, :], AF.Relu,
                                         bias=B5[:, mt:mt + 1],
                                         scale=S5[:, mt:mt + 1])
                    nc.vector.reduce_max(pg[:, mt:mt + 1], h5[:, :],
                                         axis=mybir.AxisListType.X)

                nc.sync.dma_start(out=cc_in_g[:, :], in_=pg[:, :])
                nc.gpsimd.collective_compute(
                    "AllReduce", ALU.max, replica_groups=GROUPS,
                    ins=[cc_in_g.ap()], outs=[cc_out_g.ap()])
                g_sb = pp2.tile([P, 8], F32)
                nc.sync.dma_start(out=g_sb[:, :], in_=cc_out_g[:, :])

                beta6 = pp2.tile([P, 4], F32)
                for mt in range(4):
                    ps = hpsmall.tile([P, 1], F32, tag="hsm")
                    for kk in range(8):
                        nc.tensor.matmul(
                            ps[:, :],
                            W6g[kk][:, mt * 128:(mt + 1) * 128].bitcast(mm_dt),
                            g_sb[:, kk:kk + 1].bitcast(mm_dt),
                            start=(kk == 0), stop=(kk == 7))
                    nc.vector.tensor_scalar(beta6[:, mt:mt + 1], ps[:, :],
                                            S6[:, mt:mt + 1], B6[:, mt:mt + 1],
                                            op0=ALU.mult, op1=ALU.add)

                net6 = [pp2.tile([P, HALF], F32, tag=f"net6_{i}", name=f"net6t_{i}")
                        for i in range(4)]
                for mt in range(4):
                    ps = hpsum.tile([P, HALF], F32, tag="hps")
                    for j in range(2):
                        sl = slice(j * 512, (j + 1) * 512)
                        nc.tensor.matmul(
                            ps[:, sl],
                            W6ca[:, mt * 128:(mt + 1) * 128].bitcast(mm_dt),
                            cat_a[:, sl].bitcast(mm_dt), start=True, stop=False)
                        nc.tensor.matmul(
                            ps[:, sl],
                            W6cb[:, mt * 128:(mt + 1) * 128].bitcast(mm_dt),
                            cat_b[:, sl].bitcast(mm_dt), start=False, stop=True)
                    nc.scalar.activation(net6[mt][:, :], ps[:, :], AF.Relu,
                                         bias=beta6[:, mt:mt + 1],
                                         scale=S6[:, mt:mt + 1])

                net7 = [pp2.tile([P, HALF], F32, tag=f"net7_{i}", name=f"net7t_{i}")
                        for i in range(2)]
                for mt in range(2):
                    ps = hpsum.tile([P, HALF], F32, tag="hps")
                    for j in range(2):
                        sl = slice(j * 512, (j + 1) * 512)
                        for kk in range(4):
                            nc.tensor.matmul(
                                ps[:, sl],
                                W7[kk][:, mt * 128:(mt + 1) * 128]
                                .bitcast(mm_dt),
                                net6[kk][:, sl].bitcast(mm_dt),
                                start=(kk == 0), stop=(kk == 3))
                    nc.scalar.activation(net7[mt][:, :], ps[:, :], AF.Relu,
                                         bias=B7[:, mt:mt + 1],
                                         scale=S7[:, mt:mt + 1])

                h8 = pp2.tile([P, HALF], F32)
                ps8 = hpsum.tile([P, HALF], F32, tag="hps")
                for j in range(2):
                    sl = slice(j * 512, (j + 1) * 512)
                    for kk in range(2):
                        nc.tensor.matmul(ps8[:, sl],
                                         W8[kk][:, :].bitcast(mm_dt),
                                         net7[kk][:, sl].bitcast(mm_dt),
                                         start=(kk == 0), stop=(kk == 1))
                nc.scalar.activation(h8[:, :], ps8[:, :], AF.Relu)
                lg = hpsmall.tile([1, HALF], F32, tag="hsm")
                for j in range(2):
                    sl = slice(j * 512, (j + 1) * 512)
                    nc.tensor.matmul(lg[:, sl], W9[:, :].bitcast(mm_dt),
                                     h8[:, sl].bitcast(mm_dt),
                                     start=True, stop=True)
                sc = pp2.tile([1, HALF], F32)
                nc.scalar.activation(sc[:, :], lg[:, :], AF.Sigmoid)
                nc.sync.dma_start(out=o_score[:, :], in_=sc[:, :])

                fsimT = pp2.tile([P, HALF], F32)
                psf = hpsum.tile([P, HALF], F32, tag="hps")
                for j in range(2):
                    sl = slice(j * 512, (j + 1) * 512)
                    for kk in range(2):
                        nc.tensor.matmul(psf[:, sl],
                                         W11[kk][:, :].bitcast(mm_dt),
                                         net7[kk][:, sl].bitcast(mm_dt),
                                         start=(kk == 0), stop=(kk == 1))
                nc.scalar.activation(fsimT[:, :], psf[:, :], AF.Relu)

                fsq = hb2.tile([P, HALF], F32, tag="fsq")
                nc.scalar.activation(fsq[:, :], fsimT[:, :], AF.Square)
                rrps = hpsmall.tile([1, HALF], F32, tag="hsm")
                for j in range(2):
                    nc.tensor.matmul(rrps[:, j * 512:(j + 1) * 512],
                                     ones_col[:, :],
                                     fsq[:, j * 512:(j + 1) * 512],
                                     start=True, stop=True)
                rr_own = pp2.tile([1, HALF], F32)
                nc.scalar.activation(rr_own[:, :], rrps[:, :], AF.Copy,
                                     bias=0.0, scale=1.0)

                ident = pp2.tile([P, P], F32)
                make_identity(nc, ident)
                for pt in range(8):
                    tp = hpsmall.tile([P, P], F32, tag="hsm")
                    nc.tensor.transpose(tp[:, :],
                                        fsimT[:, pt * 128:(pt + 1) * 128],
                                        ident[:, :])
                    fout = hb2.tile([P, P], F32, tag="fout")
                    nc.scalar.activation(fout[:, :], tp[:, :], AF.Copy,
                                         bias=0.0, scale=1.0)
                    nc.sync.dma_start(out=o_fsim[pt * 128:(pt + 1) * 128, :],
                                      in_=fout[:, :])

                rr_part = pp2.tile([P, 8], F32)
                for pt in range(8):
                    tp = hpsmall.tile([P, 1], F32, tag="hsm")
                    nc.tensor.transpose(tp[:, :],
                                        rr_own[:, pt * 128:(pt + 1) * 128],
                                        ident[0:1, 0:1])
                    nc.scalar.activation(rr_part[:, pt:pt + 1], tp[:, :],
                                         AF.Copy, bias=0.0, scale=1.0)

                nc.sync.dma_start(out=cc_in_fs[0:128, :], in_=fsimT[:, :])
                nc.sync.dma_start(out=cc_in_fs[128:129, :], in_=rr_own[:, :])
                nc.gpsimd.collective_compute(
                    "AllGather", ALU.bypass, replica_groups=GROUPS,
                    ins=[cc_in_fs.ap()], outs=[cc_out_fs.ap()])
                fsimTf = pp2.tile([P, N], F32)
                rr_full = pp2.tile([1, N], F32)
                for h in range(2):
                    nc.sync.dma_start(out=fsimTf[:, h * HALF:(h + 1) * HALF],
                                      in_=cc_out_fs[h, 0:128, :])
                    nc.sync.dma_start(out=rr_full[:, h * HALF:(h + 1) * HALF],
                                      in_=cc_out_fs[h, 128:129, :])
                m2f = pp2.tile([P, N], F32)
                nc.scalar.activation(m2f[:, :], fsimTf[:, :], AF.Copy,
                                     bias=0.0, scale=-2.0)

                for it in range(8):
                    sm = hb2.tile([P, N], F32, tag="sm")
                    for j in range(4):
                        sl = slice(j * 512, (j + 1) * 512)
                        ps = hpsmall.tile([P, 512], F32, tag="hsm")
                        nc.tensor.matmul(
                            ps[:, :],
                            fsimT[:, it * 128:(it + 1) * 128].bitcast(mm_dt),
                            m2f[:, sl].bitcast(mm_dt), start=True, stop=False)
                        nc.tensor.matmul(
                            ps[:, :], ones_row[:, :].bitcast(mm_dt),
                            rr_full[:, sl].bitcast(mm_dt),
                            start=False, stop=True)
                        nc.scalar.activation(sm[:, sl], ps[:, :], AF.Relu,
                                             bias=rr_part[:, it:it + 1],
                                             scale=1.0)
                    nc.sync.dma_start(out=o_simmat[it * 128:(it + 1) * 128, :],
                                      in_=sm[:, :])

    nc.finalize()
    return nc


_CACHE = {}


def _get_nc():
    if "nc" not in _CACHE:
        _CACHE["nc"] = build_nc()
    return _CACHE["nc"]


def _prep_maps(x, w1, bn1, w1_1, bn1_1, w2, bn2, w2_2, bn2_2, w3, bn3,
               w5, bn5, w6, bn6, w7, bn7, w8, w9, w11):
    def f(a):
        return np.ascontiguousarray(np.asarray(a, dtype=np.float32))

    r = 1.0 / np.sqrt(np.float32(1.0) + np.float32(EPS))

    def sb(bn, parts):
        s = (np.asarray(bn[0], np.float32) * r).astype(np.float32)
        b = np.asarray(bn[1], np.float32)
        if parts == 1:
            return f(s.reshape(-1, 1)), f(b.reshape(-1, 1))
        return f(s.reshape(parts, 128).T), f(b.reshape(parts, 128).T)

    s1v, b1v = sb(bn1, 1)
    s11v, b11v = sb(bn1_1, 1)
    s2v, b2v = sb(bn2, 1)
    s22v, b22v = sb(bn2_2, 1)
    s3v, b3v = sb(bn3, 1)
    s5v, b5v = sb(bn5, 8)
    s6v, b6v = sb(bn6, 4)
    s7v, b7v = sb(bn7, 2)

    w1 = f(w1); w2 = f(w2); w3 = f(w3)
    base = {
        "w1t": f(w1[0:6]), "w1b": f(w1[6:12] - w1[0:6]), "w1l2": f(w1_1),
        "s1": s1v, "b1": b1v, "s11": s11v, "b11": b11v,
        "w2t": f(w2[0:64]), "w2b": f(w2[64:128] - w2[0:64]), "w2l2": f(w2_2),
        "s2": s2v, "b2": b2v, "s22": s22v, "b22": b22v,
        "w3t": f(w3[0:64]), "w3b": f(w3[64:128] - w3[0:64]),
        "s3": s3v, "b3": b3v,
        "w5w": f(w5), "s5": s5v, "b5": b5v,
        "w6g": f(w6[0:1024]), "w6c": f(w6[1024:1216]),
        "s6": s6v, "b6": b6v,
        "w7w": f(w7), "s7": s7v, "b7": b7v,
        "w8w": f(w8), "w9w": f(w9), "w11sim": f(w11),
    }
    base["w11w"] = base.pop("w11sim")
    maps = []
    x = f(x)
    for c in range(8):
        b_, h_ = c // 2, c % 2
        m = dict(base)
        m["x_full"] = f(x[b_])
        m["x_own"] = f(x[b_][:, h_ * HALF:(h_ + 1) * HALF])
        maps.append(m)
    return maps


def kernel(**inputs):
    nc = _get_nc()
    maps = _prep_maps(**inputs)
    res = run_bass_kernel_spmd(nc, maps, core_ids=list(range(8)))
    rs = res.results
    center = np.zeros((B, N), np.float32)
    fsim = np.zeros((B, N, 128), np.float32)
    simmat = np.zeros((B, N, N), np.float32)
    pdist = np.zeros((B, N, N), np.float32)
    for c in range(8):
        b_, h_ = c // 2, c % 2
        sl = slice(h_ * HALF, (h_ + 1) * HALF)
        center[b_, sl] = rs[c]["o_score"][0]
        fsim[b_, sl] = rs[c]["o_fsim"]
        simmat[b_, sl] = rs[c]["o_simmat"]
        pdist[b_, sl] = rs[c]["o_pdist"]
    return center, fsim, simmat, pdist


# revision 13
# speedup vs baseline: 1.3372x; 1.3372x over previous
"""DGCNN-cls Trainium2 Bass kernel (8 NeuronCores, data-parallel batch+points).

Sharding: core c handles batch b=c//2, point rows [(c%2)*1024, (c%2)*1024+1024).
Pair collectives (AllGather features / AllReduce-max global pool) between cores
2b and 2b+1. kernel() accepts FULL inputs, returns FULL outputs.
"""

import numpy as np

import concourse.bass as bass
import concourse.bacc as bacc
import concourse.tile as tile
from concourse import mybir
from concourse.bass_utils import run_bass_kernel_spmd
from concourse.masks import make_identity

F32 = mybir.dt.float32
F32R = mybir.dt.float32r
U16 = mybir.dt.uint16
I16 = mybir.dt.int16

B = 4
C0 = 6
N = 2048
K = 20
HALF = 1024
NCHUNK = 8
P = 128
SEG = 64
NSEG = N // SEG
EPS = 1e-5
NEG = -1e30

AF = mybir.ActivationFunctionType
ALU = mybir.AluOpType


def _sap(ap, pairs, extra_off=0):
    """new AP on same tensor with custom [step,count] pairs."""
    return bass.AP(tensor=ap.tensor, offset=ap.offset + extra_off, ap=pairs)


def build_nc(mm_dt=F32, score_dt=F32):
    nc = bacc.Bacc("TRN2", target_bir_lowering=False, debug=False, num_devices=8)

    def inp(name, shape):
        return nc.dram_tensor(name, shape, F32, kind="ExternalInput")

    x_full = inp("x_full", [C0, N])
    x_own = inp("x_own", [C0, HALF])
    w1t = inp("w1t", [C0, 64]); w1b = inp("w1b", [C0, 64])
    w1l2 = inp("w1l2", [64, 64])
    s1 = inp("s1", [64, 1]); b1 = inp("b1", [64, 1])
    s11 = inp("s11", [64, 1]); b11 = inp("b11", [64, 1])
    w2t = inp("w2t", [64, 64]); w2b = inp("w2b", [64, 64])
    w2l2 = inp("w2l2", [64, 64])
    s2 = inp("s2", [64, 1]); b2 = inp("b2", [64, 1])
    s22 = inp("s22", [64, 1]); b22 = inp("b22", [64, 1])
    w3t = inp("w3t", [64, 64]); w3b = inp("w3b", [64, 64])
    s3 = inp("s3", [64, 1]); b3 = inp("b3", [64, 1])
    w5w = inp("w5w", [192, 1024])
    s5 = inp("s5", [P, 8]); b5 = inp("b5", [P, 8])
    w6g = inp("w6g", [1024, 512])
    w6c = inp("w6c", [192, 512])
    s6 = inp("s6", [P, 4]); b6 = inp("b6", [P, 4])
    w7w = inp("w7w", [512, 256])
    s7 = inp("s7", [P, 2]); b7 = inp("b7", [P, 2])
    w8w = inp("w8w", [256, 128])
    w9w = inp("w9w", [128, 1])
    w11w = inp("w11w", [256, 128])

    o_pdist = nc.dram_tensor("o_pdist", [HALF, N], F32, kind="ExternalOutput")
    o_simmat = nc.dram_tensor("o_simmat", [HALF, N], F32, kind="ExternalOutput")
    o_fsim = nc.dram_tensor("o_fsim", [HALF, 128], F32, kind="ExternalOutput")
    o_score = nc.dram_tensor("o_score", [1, HALF], F32, kind="ExternalOutput")

    idx_scr = nc.dram_tensor("idx_scr", [NCHUNK, P, 24], U16, kind="Internal")
    cc_in_x1 = nc.dram_tensor("cc_in_x1", [64, HALF], F32, kind="Internal")
    cc_out_x1 = nc.dram_tensor("cc_out_x1", [2, 64, HALF], F32, kind="Internal")
    cc_in_x2 = nc.dram_tensor("cc_in_x2", [64, HALF], F32, kind="Internal")
    cc_out_x2 = nc.dram_tensor("cc_out_x2", [2, 64, HALF], F32, kind="Internal")
    cc_in_g = nc.dram_tensor("cc_in_g", [P, 8], F32, kind="Internal")
    cc_out_g = nc.dram_tensor("cc_out_g", [P, 8], F32, kind="Internal")
    cc_in_fs = nc.dram_tensor("cc_in_fs", [129, HALF], F32, kind="Internal")
    cc_out_fs = nc.dram_tensor("cc_out_fs", [2, 129, HALF], F32, kind="Internal")
    GROUPS = [[0, 1], [2, 3], [4, 5], [6, 7]]

    with tile.TileContext(nc) as tc:
        with tc.tile_pool(name="persist", bufs=1) as pp:
            # ---------- weights ----------
            _ldn = [0]

            def load(t):
                _ldn[0] += 1
                tl = pp.tile(list(t.shape), F32, tag=f"ld{_ldn[0]}",
                             name=f"ld{_ldn[0]}")
                nc.sync.dma_start(out=tl[:, :], in_=t[:, :])
                return tl

            W1t, W1b, W1l2 = load(w1t), load(w1b), load(w1l2)
            S1, B1, S11, B11 = load(s1), load(b1), load(s11), load(b11)
            W2t, W2b, W2l2 = load(w2t), load(w2b), load(w2l2)
            S2, B2, S22, B22 = load(s2), load(b2), load(s22), load(b22)
            W3t, W3b = load(w3t), load(w3b)
            S3, B3 = load(s3), load(b3)
            W5a = pp.tile([128, 1024], F32)
            nc.sync.dma_start(out=W5a[:, :], in_=w5w[0:128, :])
            W5b = pp.tile([64, 1024], F32)
            nc.sync.dma_start(out=W5b[:, :], in_=w5w[128:192, :])
            S5, B5 = load(s5), load(b5)
            W6g = []
            for kk in range(8):
                t = pp.tile([128, 512], F32, tag=f"w6g{kk}", name=f"w6gt{kk}")
                nc.sync.dma_start(out=t[:, :], in_=w6g[kk * 128:(kk + 1) * 128, :])
                W6g.append(t)
            W6ca = pp.tile([128, 512], F32)
            nc.sync.dma_start(out=W6ca[:, :], in_=w6c[0:128, :])
            W6cb = pp.tile([64, 512], F32)
            nc.sync.dma_start(out=W6cb[:, :], in_=w6c[128:192, :])
            S6, B6 = load(s6), load(b6)
            W7 = []
            for kk in range(4):
                t = pp.tile([128, 256], F32, tag=f"w7_{kk}", name=f"w7t{kk}")
                nc.sync.dma_start(out=t[:, :], in_=w7w[kk * 128:(kk + 1) * 128, :])
                W7.append(t)
            S7, B7 = load(s7), load(b7)
            W8 = []
            for kk in range(2):
                t = pp.tile([128, 128], F32, tag=f"w8_{kk}", name=f"w8t{kk}")
                nc.sync.dma_start(out=t[:, :], in_=w8w[kk * 128:(kk + 1) * 128, :])
                W8.append(t)
            W9 = load(w9w)
            W11 = []
            for kk in range(2):
                t = pp.tile([128, 128], F32, tag=f"w11_{kk}", name=f"w11t{kk}")
                nc.sync.dma_start(out=t[:, :], in_=w11w[kk * 128:(kk + 1) * 128, :])
                W11.append(t)

            # ---------- features ----------
            f_ext = pp.tile([16, N], F32)
            nc.vector.memset(f_ext[:, :], 0.0)
            nc.sync.dma_start(out=f_ext[0:C0, :], in_=x_full[:, :])
            x_own_sb = pp.tile([C0, HALF], F32)
            nc.sync.dma_start(out=x_own_sb[:, :], in_=x_own[:, :])

            x1T = pp.tile([66, N], F32)
            x2T = pp.tile([66, N], F32)
            x1own = pp.tile([64, HALF], F32)
            x2own = pp.tile([64, HALF], F32)
            cat_a = pp.tile([P, HALF], F32)
            cat_b = pp.tile([64, HALF], F32)
            ones_col = pp.tile([P, 1], F32)
            nc.vector.memset(ones_col[:, :], 1.0)
            ones_row = pp.tile([1, P], F32)
            nc.vector.memset(ones_row[:, :], 1.0)
            ones_row_n = pp.tile([1, N], F32)
            nc.vector.memset(ones_row_n[:, :], 1.0)

            # ============ stages ============
            with (
                tc.tile_pool(name="stage", bufs=1) as sp,
                tc.tile_pool(name="sbuf2", bufs=2) as sp2,
                tc.tile_pool(name="spsum", bufs=1, space="PSUM") as spsum,
                tc.tile_pool(name="epsum", bufs=2, space="PSUM") as epsum,
            ):
                def colsq_row(dst_row_ap, src, nch, width, tagn):
                    """dst_row_ap [1,width] (any partition) = -sum_c src**2."""
                    sq = sp2.tile([64, width], F32, tag="sqtmp")
                    nc.scalar.activation(sq[0:nch, :], src, AF.Square)
                    ps = spsum.tile([1, width], F32, tag="spsum")
                    for j in range(0, width, 512):
                        nc.tensor.matmul(ps[:, j:j + 512], ones_col[0:nch, :],
                                         sq[0:nch, j:j + 512], start=True,
                                         stop=True)
                    row = sp.tile([1, width], F32, tag=tagn, name=tagn)
                    nc.scalar.activation(row[:, :], ps[:, :], AF.Copy,
                                         bias=0.0, scale=-1.0)
                    nc.sync.dma_start(out=dst_row_ap, in_=row[:, :])

                def run_stage(stage, feat_full, featC, qsrc, out_dst, out_p0):
                    CC = featC
                    KQ = CC + 2
                    if stage == 1:
                        Wt, Wb, Wl2 = W1t, W1b, W1l2
                        Sa, Ba, Sb, Bb = S1, B1, S11, B11
                    elif stage == 2:
                        Wt, Wb, Wl2 = W2t, W2b, W2l2
                        Sa, Ba, Sb, Bb = S2, B2, S22, B22
                    else:
                        Wt, Wb, Wl2 = W3t, W3b, None
                        Sa, Ba, Sb, Bb = S3, B3, None, None

                    colsq_row(feat_full[CC + 1:CC + 2, :], feat_full[0:CC, :],
                              CC, N, "nsqf")
                    nc.sync.dma_start(out=feat_full[CC:CC + 1, :],
                                      in_=ones_row_n[:, :])

                    q = sp.tile([66, HALF], F32, tag="qtile")
                    nc.scalar.activation(q[0:CC, :], qsrc, AF.Copy, bias=0.0,
                                         scale=2.0)
                    colsq_row(q[CC:CC + 1, :], qsrc, CC, HALF, "nsqq")
                    nc.sync.dma_start(out=q[CC + 1:CC + 2, :],
                                      in_=ones_row_n[:, 0:HALF])

                    idxw = sp.tile([64, NCHUNK * 160], I16, tag="idxw")

                    for m in range(NCHUNK):
                        ps = spsum.tile([P, N], F32, tag="spsum")
                        qs = q[0:KQ, m * P:(m + 1) * P]
                        for j in range(4):
                            nc.tensor.matmul(
                                ps[:, j * 512:(j + 1) * 512],
                                qs.bitcast(score_dt),
                                feat_full[0:KQ, j * 512:(j + 1) * 512]
                                .bitcast(score_dt),
                                start=True, stop=True)
                        s_sb = sp2.tile([P, N], F32, tag="s_sb")
                        nc.scalar.activation(s_sb[:, :], ps[:, :], AF.Copy,
                                             bias=0.0, scale=1.0)
                        if stage == 1:
                            d2 = sp2.tile([P, N], F32, tag="d2")
                            nc.scalar.activation(d2[:, :], ps[:, :], AF.Copy,
                                                 bias=0.0, scale=-1.0)
                            nc.sync.dma_start(
                                out=o_pdist[m * P:(m + 1) * P, :], in_=d2[:, :])

                        cands = sp.tile([P, 256], F32, tag="cands")
                        for g in range(NSEG):
                            nc.vector.max(cands[:, g * 8:(g + 1) * 8],
                                          s_sb[:, g * SEG:(g + 1) * SEG])
                        c1 = sp.tile([P, 256], F32, tag="c1")
                        c2 = sp.tile([P, 256], F32, tag="c2")
                        v8 = [sp.tile([P, 8], F32, tag=f"v8_{r}", name=f"v8t_{r}")
                              for r in range(3)]
                        nc.vector.max(v8[0][:, :], cands[:, :])
                        nc.vector.match_replace(c1[:, :], v8[0][:, :],
                                                cands[:, :], NEG)
                        nc.vector.max(v8[1][:, :], c1[:, :])
                        nc.vector.match_replace(c2[:, :], v8[1][:, :],
                                                c1[:, :], NEG)
                        nc.vector.max(v8[2][:, :], c2[:, :])
                        idx24 = sp.tile([P, 24], U16, tag="idx24")
                        for r in range(3):
                            nc.vector.max_index(idx24[:, r * 8:(r + 1) * 8],
                                                v8[r][:, :], s_sb[:, :])
                        nc.sync.dma_start(out=idx_scr[m, :, :], in_=idx24[:, :])

                    # readback wrapped idx: element j=k*128+p at [p%16, k*8+p//16]
                    ngrp = 4 if CC >= 16 else 1
                    for m in range(NCHUNK):
                        src = bass.AP(idx_scr, m * P * 24,
                                      [[24, 16], [16 * 24, 8], [1, 20]])
                        for g in range(ngrp):
                            nc.sync.dma_start(
                                out=idxw[g * 16:(g + 1) * 16,
                                         m * 160:(m + 1) * 160],
                                in_=src.bitcast(I16))

                    for m in range(NCHUNK):
                        nb = sp2.tile([64, K * P], F32, tag="nb")
                        if CC >= 16:
                            nc.gpsimd.ap_gather(
                                out_ap=nb[:, :], in_ap=feat_full[0:64, :],
                                idxs_ap=idxw[:, m * 160:(m + 1) * 160],
                                channels=64, num_elems=N, d=1, num_idxs=K * P)
                        else:
                            nc.gpsimd.ap_gather(
                                out_ap=nb[0:16, :], in_ap=f_ext[0:16, :],
                                idxs_ap=idxw[0:16, m * 160:(m + 1) * 160],
                                channels=16, num_elems=N, d=1, num_idxs=K * P)
                        h2 = sp2.tile([64, K * P], F32, tag="h2")
                        for pc in range(8):
                            sl = slice(pc * 320, (pc + 1) * 320)
                            c16 = qsrc[:, m * P + pc * 16:m * P + pc * 16 + 16]
                            ctr_b = _sap(c16, [c16.ap[0], [0, K], c16.ap[1]])
                            e1 = epsum.tile([64, 320], F32, tag="e1")
                            nc.tensor.matmul(e1[:, :], Wt[:, :].bitcast(mm_dt),
                                             nb[0:CC, sl].bitcast(mm_dt),
                                             start=True, stop=False)
                            nc.tensor.matmul(e1[:, :], Wb[:, :].bitcast(mm_dt),
                                             ctr_b.bitcast(mm_dt),
                                             start=False, stop=True)
                            if Wl2 is not None:
                                h1 = sp.tile([64, 320], F32, tag="h1")
                                nc.scalar.activation(h1[:, :], e1[:, :],
                                                     AF.Relu, bias=Ba[:, :],
                                                     scale=Sa[:, :])
                                e2 = epsum.tile([64, 320], F32, tag="e2")
                                nc.tensor.matmul(e2[:, :],
                                                 Wl2[:, :].bitcast(mm_dt),
                                                 h1[:, :].bitcast(mm_dt),
                                                 start=True, stop=True)
                                nc.scalar.activation(h2[:, sl], e2[:, :],
                                                     AF.Relu, bias=Bb[:, :],
                                                     scale=Sb[:, :])
                            else:
                                nc.scalar.activation(h2[:, sl], e1[:, :],
                                                     AF.Relu, bias=Ba[:, :],
                                                     scale=Sa[:, :])
                        h2ap = h2[:, :]
                        red_in = _sap(h2ap, [h2ap.ap[0], [320, 8], [1, 16],
                                             [16, K]])
                        nc.vector.reduce_max(
                            out_dst[out_p0:out_p0 + 64, m * P:(m + 1) * P],
                            red_in, axis=mybir.AxisListType.X)

                run_stage(1, f_ext, C0, x_own_sb[:, :], x1own, 0)

                nc.sync.dma_start(out=cc_in_x1[:, :], in_=x1own[:, :])
                nc.sync.dma_start(out=cat_a[0:64, :], in_=x1own[:, :])
                nc.gpsimd.collective_compute(
                    "AllGather", ALU.bypass, replica_groups=GROUPS,
                    ins=[cc_in_x1.ap()], outs=[cc_out_x1.ap()])
                for h in range(2):
                    nc.sync.dma_start(out=x1T[0:64, h * HALF:(h + 1) * HALF],
                                      in_=cc_out_x1[h])

                run_stage(2, x1T, 64, x1own[:, :], x2own, 0)

                nc.sync.dma_start(out=cc_in_x2[:, :], in_=x2own[:, :])
                nc.sync.dma_start(out=cat_a[64:128, :], in_=x2own[:, :])
                nc.gpsimd.collective_compute(
                    "AllGather", ALU.bypass, replica_groups=GROUPS,
                    ins=[cc_in_x2.ap()], outs=[cc_out_x2.ap()])
                for h in range(2):
                    nc.sync.dma_start(out=x2T[0:64, h * HALF:(h + 1) * HALF],
                                      in_=cc_out_x2[h])

                run_stage(3, x2T, 64, x2own[:, :], cat_b, 0)

            # ============ head ============
            with (
                tc.tile_pool(name="persist2", bufs=1) as pp2,
                tc.tile_pool(name="h2buf", bufs=2) as hb2,
                tc.tile_pool(name="hpsum", bufs=2, space="PSUM") as hpsum,
                tc.tile_pool(name="hpsmall", bufs=2, space="PSUM") as hpsmall,
            ):
                pg = pp2.tile([P, 8], F32)
                for mt in range(8):
                    ps = hpsum.tile([P, HALF], F32, tag="hps")
                    for j in range(2):
                        sl = slice(j * 512, (j + 1) * 512)
                        nc.tensor.matmul(
                            ps[:, sl],
                            W5a[:, mt * 128:(mt + 1) * 128].bitcast(mm_dt),
                            cat_a[:, sl].bitcast(mm_dt), start=True, stop=False)
                        nc.tensor.matmul(
                            ps[:, sl],
                            W5b[:, mt * 128:(mt + 1) * 128].bitcast(mm_dt),
                            cat_b[:, sl].bitcast(mm_dt), start=False, stop=True)
                    h5 = hb2.tile([P, HALF], F32, tag="h5")
                    nc.scalar.activation(h5[:, :], ps[:, :], AF.Relu,
                                         bias=B5[:, mt:mt + 1],
                                         scale=S5[:, mt:mt + 1])
                    nc.vector.reduce_max(pg[:, mt:mt + 1], h5[:, :],
                                         axis=mybir.AxisListType.X)

                nc.sync.dma_start(out=cc_in_g[:, :], in_=pg[:, :])
                nc.gpsimd.collective_compute(
                    "AllReduce", ALU.max, replica_groups=GROUPS,
                    ins=[cc_in_g.ap()], outs=[cc_out_g.ap()])
                g_sb = pp2.tile([P, 8], F32)
                nc.sync.dma_start(out=g_sb[:, :], in_=cc_out_g[:, :])

                beta6 = pp2.tile([P, 4], F32)
                for mt in range(4):
                    ps = hpsmall.tile([P, 1], F32, tag="hsm")
                    for kk in range(8):
                        nc.tensor.matmul(
                            ps[:, :],
                            W6g[kk][:, mt * 128:(mt + 1) * 128].bitcast(mm_dt),
                            g_sb[:, kk:kk + 1].bitcast(mm_dt),
                            start=(kk == 0), stop=(kk == 7))
                    nc.vector.tensor_scalar(beta6[:, mt:mt + 1], ps[:, :],
                                            S6[:, mt:mt + 1], B6[:, mt:mt + 1],
                                            op0=ALU.mult, op1=ALU.add)

                net6 = [pp2.tile([P, HALF], F32, tag=f"net6_{i}", name=f"net6t_{i}")
                        for i in range(4)]
                for mt in range(4):
                    ps = hpsum.tile([P, HALF], F32, tag="hps")
                    for j in range(2):
                        sl = slice(j * 512, (j + 1) * 512)
                        nc.tensor.matmul(
                            ps[:, sl],
                            W6ca[:, mt * 128:(mt + 1) * 128].bitcast(mm_dt),
                            cat_a[:, sl].bitcast(mm_dt), start=True, stop=False)
                        nc.tensor.matmul(
                            ps[:, sl],
                            W6cb[:, mt * 128:(mt + 1) * 128].bitcast(mm_dt),
                            cat_b[:, sl].bitcast(mm_dt), start=False, stop=True)
                    nc.scalar.activation(net6[mt][:, :], ps[:, :], AF.Relu,
                                         bias=beta6[:, mt:mt + 1],
                                         scale=S6[:, mt:mt + 1])

                net7 = [pp2.tile([P, HALF], F32, tag=f"net7_{i}", name=f"net7t_{i}")
                        for i in range(2)]
                for mt in range(2):
                    ps = hpsum.tile([P, HALF], F32, tag="hps")
                    for j in range(2):
                        sl = slice(j * 512, (j + 1) * 512)
                        for kk in range(4):
                            nc.tensor.matmul(
                                ps[:, sl],
                                W7[kk][:, mt * 128:(mt + 1) * 128]
                                .bitcast(mm_dt),
                                net6[kk][:, sl].bitcast(mm_dt),
                                start=(kk == 0), stop=(kk == 3))
                    nc.scalar.activation(net7[mt][:, :], ps[:, :], AF.Relu,
                                         bias=B7[:, mt:mt + 1],
                                         scale=S7[:, mt:mt + 1])

                h8 = pp2.tile([P, HALF], F32)
                ps8 = hpsum.tile([P, HALF], F32, tag="hps")
                for j in range(2):
                    sl = slice(j * 512, (j + 1) * 512)
                    for kk in range(2):
                        nc.tensor.matmul(ps8[:, sl],
                                         W8[kk][:, :].bitcast(mm_dt),
                                         net7[kk][:, sl].bitcast(mm_dt),
                                         start=(kk == 0), stop=(kk == 1))
                nc.scalar.activation(h8[:, :], ps8[:, :], AF.Relu)
                lg = hpsmall.tile([1, HALF], F32, tag="hsm")
                for j in range(2):
                    sl = slice(j * 512, (j + 1) * 512)
                    nc.tensor.matmul(lg[:, sl], W9[:, :].bitcast(mm_dt),
                                     h8[:, sl].bitcast(mm_dt),
                                     start=True, stop=True)
                sc = pp2.tile([1, HALF], F32)
                nc.scalar.activation(sc[:, :], lg[:, :], AF.Sigmoid)
                nc.sync.dma_start(out=o_score[:, :], in_=sc[:, :])

                fsimT = pp2.tile([P, HALF], F32)
                psf = hpsum.tile([P, HALF], F32, tag="hps")
                for j in range(2):
                    sl = slice(j * 512, (j + 1) * 512)
                    for kk in range(2):
                        nc.tensor.matmul(psf[:, sl],
                                         W11[kk][:, :].bitcast(mm_dt),
                                         net7[kk][:, sl].bitcast(mm_dt),
                                         start=(kk == 0), stop=(kk == 1))
                nc.scalar.activation(fsimT[:, :], psf[:, :], AF.Relu)

                fsq = hb2.tile([P, HALF], F32, tag="fsq")
                nc.scalar.activation(fsq[:, :], fsimT[:, :], AF.Square)
                rrps = hpsmall.tile([1, HALF], F32, tag="hsm")
                for j in range(2):
                    nc.tensor.matmul(rrps[:, j * 512:(j + 1) * 512],
                                     ones_col[:, :],
                                     fsq[:, j * 512:(j + 1) * 512],
                                     start=True, stop=True)
                rr_own = pp2.tile([1, HALF], F32)
                nc.scalar.activation(rr_own[:, :], rrps[:, :], AF.Copy,
                                     bias=0.0, scale=1.0)

                ident = pp2.tile([P, P], F32)
                make_identity(nc, ident)
                for pt in range(8):
                    tp = hpsmall.tile([P, P], F32, tag="hsm")
                    nc.tensor.transpose(tp[:, :],
                                        fsimT[:, pt * 128:(pt + 1) * 128],
                                        ident[:, :])
                    fout = hb2.tile([P, P], F32, tag="fout")
                    nc.scalar.activation(fout[:, :], tp[:, :], AF.Copy,
                                         bias=0.0, scale=1.0)
                    nc.sync.dma_start(out=o_fsim[pt * 128:(pt + 1) * 128, :],
                                      in_=fout[:, :])

                rr_part = pp2.tile([P, 8], F32)
                for pt in range(8):
                    tp = hpsmall.tile([P, 1], F32, tag="hsm")
                    nc.tensor.transpose(tp[:, :],
                                        rr_own[:, pt * 128:(pt + 1) * 128],
                                        ident[0:1, 0:1])
                    nc.scalar.activation(rr_part[:, pt:pt + 1], tp[:, :],
                                         AF.Copy, bias=0.0, scale=1.0)

                nc.sync.dma_start(out=cc_in_fs[0:128, :], in_=fsimT[:, :])
                nc.sync.dma_start(out=cc_in_fs[128:129, :], in_=rr_own[:, :])
                nc.gpsimd.collective_compute(
                    "AllGather", ALU.bypass, replica_groups=GROUPS,
                    ins=[cc_in_fs.ap()], outs=[cc_out_fs.ap()])
                fsimTf = pp2.tile([P, N], F32)
                rr_full = pp2.tile([1, N], F32)
                for h in range(2):
                    nc.sync.dma_start(out=fsimTf[:, h * HALF:(h + 1) * HALF],
                                      in_=cc_out_fs[h, 0:128, :])
                    nc.sync.dma_start(out=rr_full[:, h * HALF:(h + 1) * HALF],
                                      in_=cc_out_fs[h, 128:129, :])
                m2f = pp2.tile([P, N], F32)
                nc.scalar.activation(m2f[:, :], fsimTf[:, :], AF.Copy,
                                     bias=0.0, scale=-2.0)

                for it in range(8):
                    sm = hb2.tile([P, N], F32, tag="sm")
                    for j in range(4):
                        sl = slice(j * 512, (j + 1) * 512)
                        ps = hpsmall.tile([P, 512], F32, tag="hsm")
                        nc.tensor.matmul(
                            ps[:, :],
                            fsimT[:, it * 128:(it + 1) * 128].bitcast(mm_dt),
                            m2f[:, sl].bitcast(mm_dt), start=True, stop=False)
                        nc.tensor.matmul(
                            ps[:, :], ones_row[:, :].bitcast(mm_dt),
                            rr_full[:, sl].bitcast(mm_dt),
                            start=False, stop=True)
                        nc.scalar.activation(sm[:, sl], ps[:, :], AF.Relu,
                                             bias=rr_part[:, it:it + 1],
                                             scale=1.0)
                    nc.sync.dma_start(out=o_simmat[it * 128:(it + 1) * 128, :],
                                      in_=sm[:, :])

    nc.finalize()
    return nc


_CACHE = {}


def _get_nc():
    if "nc" not in _CACHE:
        _CACHE["nc"] = build_nc()
    return _CACHE["nc"]


def _prep_maps(x, w1, bn1, w1_1, bn1_1, w2, bn2, w2_2, bn2_2, w3, bn3,
               w5, bn5, w6, bn6, w7, bn7, w8, w9, w11):
    def f(a):
        return np.ascontiguousarray(np.asarray(a, dtype=np.float32))

    r = 1.0 / np.sqrt(np.float32(1.0) + np.float32(EPS))

    def sb(bn, parts):
        s = (np.asarray(bn[0], np.float32) * r).astype(np.float32)
        b = np.asarray(bn[1], np.float32)
        if parts == 1:
            return f(s.reshape(-1, 1)), f(b.reshape(-1, 1))
        return f(s.reshape(parts, 128).T), f(b.reshape(parts, 128).T)

    s1v, b1v = sb(bn1, 1)
    s11v, b11v = sb(bn1_1, 1)
    s2v, b2v = sb(bn2, 1)
    s22v, b22v = sb(bn2_2, 1)
    s3v, b3v = sb(bn3, 1)
    s5v, b5v = sb(bn5, 8)
    s6v, b6v = sb(bn6, 4)
    s7v, b7v = sb(bn7, 2)

    w1 = f(w1); w2 = f(w2); w3 = f(w3)
    base = {
        "w1t": f(w1[0:6]), "w1b": f(w1[6:12] - w1[0:6]), "w1l2": f(w1_1),
        "s1": s1v, "b1": b1v, "s11": s11v, "b11": b11v,
        "w2t": f(w2[0:64]), "w2b": f(w2[64:128] - w2[0:64]), "w2l2": f(w2_2),
        "s2": s2v, "b2": b2v, "s22": s22v, "b22": b22v,
        "w3t": f(w3[0:64]), "w3b": f(w3[64:128] - w3[0:64]),
        "s3": s3v, "b3": b3v,
        "w5w": f(w5), "s5": s5v, "b5": b5v,
        "w6g": f(w6[0:1024]), "w6c": f(w6[1024:1216]),
        "s6": s6v, "b6": b6v,
        "w7w": f(w7), "s7": s7v, "b7": b7v,
        "w8w": f(w8), "w9w": f(w9), "w11sim": f(w11),
    }
    base["w11w"] = base.pop("w11sim")
    maps = []
    x = f(x)
    for c in range(8):
        b_, h_ = c // 2, c % 2
        m = dict(base)
        m["x_full"] = f(x[b_])
        m["x_own"] = f(x[b_][:, h_ * HALF:(h_ + 1) * HALF])
        maps.append(m)
    return maps


def _get_runner():
    """Cached jitted SPMD callable (mirrors bass2jax.run_bass_via_pjrt but
    keeps the compiled executable across kernel() calls)."""
    if "runner" in _CACHE:
        return _CACHE["runner"]
    import jax
    from jax.sharding import Mesh, PartitionSpec
    from jax.experimental.shard_map import shard_map
    from concourse import bass2jax
    from concourse import mybir as _mb

    nc = _get_nc()
    bass2jax.install_neuronx_cc_hook()
    partition_name = (nc.partition_id_tensor.name
                      if nc.partition_id_tensor else None)
    in_names, out_names, out_avals, zero_outs = [], [], [], []
    for alloc in nc.m.functions[0].allocations:
        if not isinstance(alloc, _mb.MemoryLocationSet):
            continue
        name = alloc.memorylocations[0].name
        if alloc.kind == "ExternalInput":
            if name != partition_name:
                in_names.append(name)
        elif alloc.kind == "ExternalOutput":
            out_names.append(name)
            shape = tuple(alloc.tensor_shape)
            dtype = _mb.dt.np(alloc.dtype)
            out_avals.append(jax.core.ShapedArray(shape, dtype))
            zero_outs.append(np.zeros(shape, dtype))
    n_params = len(in_names)
    n_outs = len(out_avals)
    all_in = list(in_names) + list(out_names)
    if partition_name is not None:
        all_in.append(partition_name)
    donate = tuple(range(n_params, n_params + n_outs))

    def _body(*args):
        operands = list(args)
        if partition_name is not None:
            operands.append(bass2jax.partition_id_tensor())
        outs = bass2jax._bass_exec_p.bind(
            *operands, out_avals=tuple(out_avals), in_names=tuple(all_in),
            out_names=tuple(out_names), lowering_input_output_aliases=(),
            sim_require_finite=True, sim_require_nnan=True, nc=nc)
        return tuple(outs)

    devices = jax.devices()[:8]
    mesh = Mesh(np.asarray(devices), ("core",))
    in_specs = (PartitionSpec("core"),) * (n_params + n_outs)
    out_specs = (PartitionSpec("core"),) * n_outs
    sharded = jax.jit(
        shard_map(_body, mesh=mesh, in_specs=in_specs, out_specs=out_specs,
                  check_rep=False),
        donate_argnums=donate, keep_unused=True)

    def run(maps):
        per_core = [[np.asarray(m[name]) for name in in_names[:n_params]]
                    for m in maps]
        concat_in = [np.concatenate([per_core[c][i] for c in range(8)], axis=0)
                     for i in range(n_params)]
        concat_zeros = [np.zeros((8 * z.shape[0], *z.shape[1:]), z.dtype)
                        for z in zero_outs]
        out_arrs = sharded(*concat_in, *concat_zeros)
        return [
            {name: np.asarray(out_arrs[i]).reshape(8, *out_avals[i].shape)[c]
             for i, name in enumerate(out_names)}
            for c in range(8)
        ]

    _CACHE["runner"] = run
    return run


def kernel(**inputs):
    maps = _prep_maps(**inputs)
    rs = _get_runner()(maps)
    center = np.zeros((B, N), np.float32)
    fsim = np.zeros((B, N, 128), np.float32)
    simmat = np.zeros((B, N, N), np.float32)
    pdist = np.zeros((B, N, N), np.float32)
    for c in range(8):
        b_, h_ = c // 2, c % 2
        sl = slice(h_ * HALF, (h_ + 1) * HALF)
        center[b_, sl] = rs[c]["o_score"][0]
        fsim[b_, sl] = rs[c]["o_fsim"]
        simmat[b_, sl] = rs[c]["o_simmat"]
        pdist[b_, sl] = rs[c]["o_pdist"]
    return center, fsim, simmat, pdist


# revision 16
# speedup vs baseline: 72.6263x; 54.3131x over previous
"""DGCNN-cls Trainium2 Bass kernel (8 NeuronCores, data-parallel batch+points).

Sharding: core c handles batch b=c//2, point rows [(c%2)*1024, (c%2)*1024+1024).
Pair collectives (AllGather features / AllReduce-max global pool) between cores
2b and 2b+1. kernel() accepts FULL inputs, returns FULL outputs.
"""

import numpy as np

import concourse.bass as bass
import concourse.bacc as bacc
import concourse.tile as tile
from concourse import mybir
from concourse.bass_utils import run_bass_kernel_spmd
from concourse.masks import make_identity

F32 = mybir.dt.float32
F32R = mybir.dt.float32r
U16 = mybir.dt.uint16
I16 = mybir.dt.int16

B = 4
C0 = 6
N = 2048
K = 20
HALF = 1024
NCHUNK = 8
P = 128
SEG = 64
NSEG = N // SEG
EPS = 1e-5
NEG = -1e30

AF = mybir.ActivationFunctionType
ALU = mybir.AluOpType


def _sap(ap, pairs, extra_off=0):
    """new AP on same tensor with custom [step,count] pairs."""
    return bass.AP(tensor=ap.tensor, offset=ap.offset + extra_off, ap=pairs)


def build_nc(mm_dt=F32, score_dt=F32):
    nc = bacc.Bacc("TRN2", target_bir_lowering=False, debug=False, num_devices=8)

    def inp(name, shape):
        return nc.dram_tensor(name, shape, F32, kind="ExternalInput")

    x_full = inp("x_full", [C0, N])
    x_own = inp("x_own", [C0, HALF])
    w1t = inp("w1t", [C0, 64]); w1b = inp("w1b", [C0, 64])
    w1l2 = inp("w1l2", [64, 64])
    s1 = inp("s1", [64, 1]); b1 = inp("b1", [64, 1])
    s11 = inp("s11", [64, 1]); b11 = inp("b11", [64, 1])
    w2t = inp("w2t", [64, 64]); w2b = inp("w2b", [64, 64])
    w2l2 = inp("w2l2", [64, 64])
    s2 = inp("s2", [64, 1]); b2 = inp("b2", [64, 1])
    s22 = inp("s22", [64, 1]); b22 = inp("b22", [64, 1])
    w3t = inp("w3t", [64, 64]); w3b = inp("w3b", [64, 64])
    s3 = inp("s3", [64, 1]); b3 = inp("b3", [64, 1])
    w5w = inp("w5w", [192, 1024])
    s5 = inp("s5", [P, 8]); b5 = inp("b5", [P, 8])
    w6g = inp("w6g", [1024, 512])
    w6c = inp("w6c", [192, 512])
    s6 = inp("s6", [P, 4]); b6 = inp("b6", [P, 4])
    w7w = inp("w7w", [512, 256])
    s7 = inp("s7", [P, 2]); b7 = inp("b7", [P, 2])
    w8w = inp("w8w", [256, 128])
    w9w = inp("w9w", [128, 1])
    w11w = inp("w11w", [256, 128])

    o_pdist = nc.dram_tensor("o_pdist", [HALF, N], F32, kind="ExternalOutput")
    o_simmat = nc.dram_tensor("o_simmat", [HALF, N], F32, kind="ExternalOutput")
    o_fsim = nc.dram_tensor("o_fsim", [HALF, 128], F32, kind="ExternalOutput")
    o_score = nc.dram_tensor("o_score", [1, HALF], F32, kind="ExternalOutput")

    idx_scr = nc.dram_tensor("idx_scr", [NCHUNK, P, 24], U16, kind="Internal")
    cc_in_x1 = nc.dram_tensor("cc_in_x1", [64, HALF], F32, kind="Internal")
    cc_out_x1 = nc.dram_tensor("cc_out_x1", [2, 64, HALF], F32, kind="Internal")
    cc_in_x2 = nc.dram_tensor("cc_in_x2", [64, HALF], F32, kind="Internal")
    cc_out_x2 = nc.dram_tensor("cc_out_x2", [2, 64, HALF], F32, kind="Internal")
    cc_in_g = nc.dram_tensor("cc_in_g", [P, 8], F32, kind="Internal")
    cc_out_g = nc.dram_tensor("cc_out_g", [P, 8], F32, kind="Internal")
    cc_in_fs = nc.dram_tensor("cc_in_fs", [129, HALF], F32, kind="Internal")
    cc_out_fs = nc.dram_tensor("cc_out_fs", [2, 129, HALF], F32, kind="Internal")
    GROUPS = [[0, 1], [2, 3], [4, 5], [6, 7]]

    with tile.TileContext(nc) as tc:
        with tc.tile_pool(name="persist", bufs=1) as pp:
            # ---------- weights ----------
            _ldn = [0]

            def load(t):
                _ldn[0] += 1
                tl = pp.tile(list(t.shape), F32, tag=f"ld{_ldn[0]}",
                             name=f"ld{_ldn[0]}")
                nc.sync.dma_start(out=tl[:, :], in_=t[:, :])
                return tl

            W1t, W1b, W1l2 = load(w1t), load(w1b), load(w1l2)
            S1, B1, S11, B11 = load(s1), load(b1), load(s11), load(b11)
            W2t, W2b, W2l2 = load(w2t), load(w2b), load(w2l2)
            S2, B2, S22, B22 = load(s2), load(b2), load(s22), load(b22)
            W3t, W3b = load(w3t), load(w3b)
            S3, B3 = load(s3), load(b3)
            W5a = pp.tile([128, 1024], F32)
            nc.sync.dma_start(out=W5a[:, :], in_=w5w[0:128, :])
            W5b = pp.tile([64, 1024], F32)
            nc.sync.dma_start(out=W5b[:, :], in_=w5w[128:192, :])
            S5, B5 = load(s5), load(b5)
            W6g = []
            for kk in range(8):
                t = pp.tile([128, 512], F32, tag=f"w6g{kk}", name=f"w6gt{kk}")
                nc.sync.dma_start(out=t[:, :], in_=w6g[kk * 128:(kk + 1) * 128, :])
                W6g.append(t)
            W6ca = pp.tile([128, 512], F32)
            nc.sync.dma_start(out=W6ca[:, :], in_=w6c[0:128, :])
            W6cb = pp.tile([64, 512], F32)
            nc.sync.dma_start(out=W6cb[:, :], in_=w6c[128:192, :])
            S6, B6 = load(s6), load(b6)
            W7 = []
            for kk in range(4):
                t = pp.tile([128, 256], F32, tag=f"w7_{kk}", name=f"w7t{kk}")
                nc.sync.dma_start(out=t[:, :], in_=w7w[kk * 128:(kk + 1) * 128, :])
                W7.append(t)
            S7, B7 = load(s7), load(b7)
            W8 = []
            for kk in range(2):
                t = pp.tile([128, 128], F32, tag=f"w8_{kk}", name=f"w8t{kk}")
                nc.sync.dma_start(out=t[:, :], in_=w8w[kk * 128:(kk + 1) * 128, :])
                W8.append(t)
            W9 = load(w9w)
            W11 = []
            for kk in range(2):
                t = pp.tile([128, 128], F32, tag=f"w11_{kk}", name=f"w11t{kk}")
                nc.sync.dma_start(out=t[:, :], in_=w11w[kk * 128:(kk + 1) * 128, :])
                W11.append(t)

            # ---------- features ----------
            f_ext = pp.tile([16, N], F32)
            nc.vector.memset(f_ext[:, :], 0.0)
            nc.sync.dma_start(out=f_ext[0:C0, :], in_=x_full[:, :])
            x_own_sb = pp.tile([C0, HALF], F32)
            nc.sync.dma_start(out=x_own_sb[:, :], in_=x_own[:, :])

            x1T = pp.tile([66, N], F32)
            x2T = pp.tile([66, N], F32)
            x1own = pp.tile([64, HALF], F32)
            x2own = pp.tile([64, HALF], F32)
            cat_a = pp.tile([P, HALF], F32)
            cat_b = pp.tile([64, HALF], F32)
            ones_col = pp.tile([P, 1], F32)
            nc.vector.memset(ones_col[:, :], 1.0)
            ones_row = pp.tile([1, P], F32)
            nc.vector.memset(ones_row[:, :], 1.0)
            ones_row_n = pp.tile([1, N], F32)
            nc.vector.memset(ones_row_n[:, :], 1.0)

            # ============ stages ============
            with (
                tc.tile_pool(name="stage", bufs=1) as sp,
                tc.tile_pool(name="sbuf2", bufs=2) as sp2,
                tc.tile_pool(name="spsum", bufs=1, space="PSUM") as spsum,
                tc.tile_pool(name="epsum", bufs=2, space="PSUM") as epsum,
            ):
                def colsq_row(dst_row_ap, src, nch, width, tagn):
                    """dst_row_ap [1,width] (any partition) = -sum_c src**2."""
                    sq = sp2.tile([64, width], F32, tag="sqtmp")
                    nc.scalar.activation(sq[0:nch, :], src, AF.Square)
                    ps = spsum.tile([1, width], F32, tag="spsum")
                    for j in range(0, width, 512):
                        nc.tensor.matmul(ps[:, j:j + 512], ones_col[0:nch, :],
                                         sq[0:nch, j:j + 512], start=True,
                                         stop=True)
                    row = sp.tile([1, width], F32, tag=tagn, name=tagn)
                    nc.scalar.activation(row[:, :], ps[:, :], AF.Copy,
                                         bias=0.0, scale=-1.0)
                    nc.sync.dma_start(out=dst_row_ap, in_=row[:, :])

                def run_stage(stage, feat_full, featC, qsrc, out_dst, out_p0):
                    CC = featC
                    KQ = CC + 2
                    if stage == 1:
                        Wt, Wb, Wl2 = W1t, W1b, W1l2
                        Sa, Ba, Sb, Bb = S1, B1, S11, B11
                    elif stage == 2:
                        Wt, Wb, Wl2 = W2t, W2b, W2l2
                        Sa, Ba, Sb, Bb = S2, B2, S22, B22
                    else:
                        Wt, Wb, Wl2 = W3t, W3b, None
                        Sa, Ba, Sb, Bb = S3, B3, None, None

                    colsq_row(feat_full[CC + 1:CC + 2, :], feat_full[0:CC, :],
                              CC, N, "nsqf")
                    nc.sync.dma_start(out=feat_full[CC:CC + 1, :],
                                      in_=ones_row_n[:, :])

                    q = sp.tile([66, HALF], F32, tag="qtile")
                    nc.scalar.activation(q[0:CC, :], qsrc, AF.Copy, bias=0.0,
                                         scale=2.0)
                    colsq_row(q[CC:CC + 1, :], qsrc, CC, HALF, "nsqq")
                    nc.sync.dma_start(out=q[CC + 1:CC + 2, :],
                                      in_=ones_row_n[:, 0:HALF])

                    idxw = sp.tile([64, NCHUNK * 160], I16, tag="idxw")

                    for m in range(NCHUNK):
                        ps = spsum.tile([P, N], F32, tag="spsum")
                        qs = q[0:KQ, m * P:(m + 1) * P]
                        for j in range(4):
                            nc.tensor.matmul(
                                ps[:, j * 512:(j + 1) * 512],
                                qs.bitcast(score_dt),
                                feat_full[0:KQ, j * 512:(j + 1) * 512]
                                .bitcast(score_dt),
                                start=True, stop=True)
                        s_sb = sp2.tile([P, N], F32, tag="s_sb")
                        nc.scalar.activation(s_sb[:, :], ps[:, :], AF.Copy,
                                             bias=0.0, scale=1.0)
                        if stage == 1:
                            d2 = sp2.tile([P, N], F32, tag="d2")
                            nc.scalar.activation(d2[:, :], ps[:, :], AF.Copy,
                                                 bias=0.0, scale=-1.0)
                            nc.sync.dma_start(
                                out=o_pdist[m * P:(m + 1) * P, :], in_=d2[:, :])

                        cands = sp.tile([P, 256], F32, tag="cands")
                        for g in range(NSEG):
                            nc.vector.max(cands[:, g * 8:(g + 1) * 8],
                                          s_sb[:, g * SEG:(g + 1) * SEG])
                        c1 = sp.tile([P, 256], F32, tag="c1")
                        c2 = sp.tile([P, 256], F32, tag="c2")
                        v8 = [sp.tile([P, 8], F32, tag=f"v8_{r}", name=f"v8t_{r}")
                              for r in range(3)]
                        nc.vector.max(v8[0][:, :], cands[:, :])
                        nc.vector.match_replace(c1[:, :], v8[0][:, :],
                                                cands[:, :], NEG)
                        nc.vector.max(v8[1][:, :], c1[:, :])
                        nc.vector.match_replace(c2[:, :], v8[1][:, :],
                                                c1[:, :], NEG)
                        nc.vector.max(v8[2][:, :], c2[:, :])
                        idx24 = sp.tile([P, 24], U16, tag="idx24")
                        for r in range(3):
                            nc.vector.max_index(idx24[:, r * 8:(r + 1) * 8],
                                                v8[r][:, :], s_sb[:, :])
                        nc.sync.dma_start(out=idx_scr[m, :, :], in_=idx24[:, :])

                    # readback wrapped idx: element j=k*128+p at [p%16, k*8+p//16]
                    ngrp = 4 if CC >= 16 else 1
                    for m in range(NCHUNK):
                        src = bass.AP(idx_scr, m * P * 24,
                                      [[24, 16], [16 * 24, 8], [1, 20]])
                        for g in range(ngrp):
                            nc.sync.dma_start(
                                out=idxw[g * 16:(g + 1) * 16,
                                         m * 160:(m + 1) * 160],
                                in_=src.bitcast(I16))

                    for m in range(NCHUNK):
                        nb = sp2.tile([64, K * P], F32, tag="nb")
                        if CC >= 16:
                            nc.gpsimd.ap_gather(
                                out_ap=nb[:, :], in_ap=feat_full[0:64, :],
                                idxs_ap=idxw[:, m * 160:(m + 1) * 160],
                                channels=64, num_elems=N, d=1, num_idxs=K * P)
                        else:
                            nc.gpsimd.ap_gather(
                                out_ap=nb[0:16, :], in_ap=f_ext[0:16, :],
                                idxs_ap=idxw[0:16, m * 160:(m + 1) * 160],
                                channels=16, num_elems=N, d=1, num_idxs=K * P)
                        h2 = sp2.tile([64, K * P], F32, tag="h2")
                        for pc in range(8):
                            sl = slice(pc * 320, (pc + 1) * 320)
                            c16 = qsrc[:, m * P + pc * 16:m * P + pc * 16 + 16]
                            ctr_b = _sap(c16, [c16.ap[0], [0, K], c16.ap[1]])
                            e1 = epsum.tile([64, 320], F32, tag="e1")
                            nc.tensor.matmul(e1[:, :], Wt[:, :].bitcast(mm_dt),
                                             nb[0:CC, sl].bitcast(mm_dt),
                                             start=True, stop=False)
                            nc.tensor.matmul(e1[:, :], Wb[:, :].bitcast(mm_dt),
                                             ctr_b.bitcast(mm_dt),
                                             start=False, stop=True)
                            if Wl2 is not None:
                                h1 = sp.tile([64, 320], F32, tag="h1")
                                nc.scalar.activation(h1[:, :], e1[:, :],
                                                     AF.Relu, bias=Ba[:, :],
                                                     scale=Sa[:, :])
                                e2 = epsum.tile([64, 320], F32, tag="e2")
                                nc.tensor.matmul(e2[:, :],
                                                 Wl2[:, :].bitcast(mm_dt),
                                                 h1[:, :].bitcast(mm_dt),
                                                 start=True, stop=True)
                                nc.scalar.activation(h2[:, sl], e2[:, :],
                                                     AF.Relu, bias=Bb[:, :],
                                                     scale=Sb[:, :])
                            else:
                                nc.scalar.activation(h2[:, sl], e1[:, :],
                                                     AF.Relu, bias=Ba[:, :],
                                                     scale=Sa[:, :])
                        h2ap = h2[:, :]
                        red_in = _sap(h2ap, [h2ap.ap[0], [320, 8], [1, 16],
                                             [16, K]])
                        nc.vector.reduce_max(
                            out_dst[out_p0:out_p0 + 64, m * P:(m + 1) * P],
                            red_in, axis=mybir.AxisListType.X)

                run_stage(1, f_ext, C0, x_own_sb[:, :], x1own, 0)

                nc.sync.dma_start(out=cc_in_x1[:, :], in_=x1own[:, :])
                nc.sync.dma_start(out=cat_a[0:64, :], in_=x1own[:, :])
                nc.gpsimd.collective_compute(
                    "AllGather", ALU.bypass, replica_groups=GROUPS,
                    ins=[cc_in_x1.ap()], outs=[cc_out_x1.ap()])
                for h in range(2):
                    nc.sync.dma_start(out=x1T[0:64, h * HALF:(h + 1) * HALF],
                                      in_=cc_out_x1[h])

                run_stage(2, x1T, 64, x1own[:, :], x2own, 0)

                nc.sync.dma_start(out=cc_in_x2[:, :], in_=x2own[:, :])
                nc.sync.dma_start(out=cat_a[64:128, :], in_=x2own[:, :])
                nc.gpsimd.collective_compute(
                    "AllGather", ALU.bypass, replica_groups=GROUPS,
                    ins=[cc_in_x2.ap()], outs=[cc_out_x2.ap()])
                for h in range(2):
                    nc.sync.dma_start(out=x2T[0:64, h * HALF:(h + 1) * HALF],
                                      in_=cc_out_x2[h])

                run_stage(3, x2T, 64, x2own[:, :], cat_b, 0)

            # ============ head ============
            with (
                tc.tile_pool(name="persist2", bufs=1) as pp2,
                tc.tile_pool(name="h2buf", bufs=2) as hb2,
                tc.tile_pool(name="hpsum", bufs=2, space="PSUM") as hpsum,
                tc.tile_pool(name="hpsmall", bufs=2, space="PSUM") as hpsmall,
            ):
                pg = pp2.tile([P, 8], F32)
                for mt in range(8):
                    ps = hpsum.tile([P, HALF], F32, tag="hps")
                    for j in range(2):
                        sl = slice(j * 512, (j + 1) * 512)
                        nc.tensor.matmul(
                            ps[:, sl],
                            W5a[:, mt * 128:(mt + 1) * 128].bitcast(mm_dt),
                            cat_a[:, sl].bitcast(mm_dt), start=True, stop=False)
                        nc.tensor.matmul(
                            ps[:, sl],
                            W5b[:, mt * 128:(mt + 1) * 128].bitcast(mm_dt),
                            cat_b[:, sl].bitcast(mm_dt), start=False, stop=True)
                    h5 = hb2.tile([P, HALF], F32, tag="h5")
                    nc.scalar.activation(h5[:, :], ps[:, :], AF.Relu,
                                         bias=B5[:, mt:mt + 1],
                                         scale=S5[:, mt:mt + 1])
                    nc.vector.reduce_max(pg[:, mt:mt + 1], h5[:, :],
                                         axis=mybir.AxisListType.X)

                nc.sync.dma_start(out=cc_in_g[:, :], in_=pg[:, :])
                nc.gpsimd.collective_compute(
                    "AllReduce", ALU.max, replica_groups=GROUPS,
                    ins=[cc_in_g.ap()], outs=[cc_out_g.ap()])
                g_sb = pp2.tile([P, 8], F32)
                nc.sync.dma_start(out=g_sb[:, :], in_=cc_out_g[:, :])

                beta6 = pp2.tile([P, 4], F32)
                for mt in range(4):
                    ps = hpsmall.tile([P, 1], F32, tag="hsm")
                    for kk in range(8):
                        nc.tensor.matmul(
                            ps[:, :],
                            W6g[kk][:, mt * 128:(mt + 1) * 128].bitcast(mm_dt),
                            g_sb[:, kk:kk + 1].bitcast(mm_dt),
                            start=(kk == 0), stop=(kk == 7))
                    nc.vector.tensor_scalar(beta6[:, mt:mt + 1], ps[:, :],
                                            S6[:, mt:mt + 1], B6[:, mt:mt + 1],
                                            op0=ALU.mult, op1=ALU.add)

                net6 = [pp2.tile([P, HALF], F32, tag=f"net6_{i}", name=f"net6t_{i}")
                        for i in range(4)]
                for mt in range(4):
                    ps = hpsum.tile([P, HALF], F32, tag="hps")
                    for j in range(2):
                        sl = slice(j * 512, (j + 1) * 512)
                        nc.tensor.matmul(
                            ps[:, sl],
                            W6ca[:, mt * 128:(mt + 1) * 128].bitcast(mm_dt),
                            cat_a[:, sl].bitcast(mm_dt), start=True, stop=False)
                        nc.tensor.matmul(
                            ps[:, sl],
                            W6cb[:, mt * 128:(mt + 1) * 128].bitcast(mm_dt),
                            cat_b[:, sl].bitcast(mm_dt), start=False, stop=True)
                    nc.scalar.activation(net6[mt][:, :], ps[:, :], AF.Relu,
                                         bias=beta6[:, mt:mt + 1],
                                         scale=S6[:, mt:mt + 1])

                net7 = [pp2.tile([P, HALF], F32, tag=f"net7_{i}", name=f"net7t_{i}")
                        for i in range(2)]
                for mt in range(2):
                    ps = hpsum.tile([P, HALF], F32, tag="hps")
                    for j in range(2):
                        sl = slice(j * 512, (j + 1) * 512)
                        for kk in range(4):
                            nc.tensor.matmul(
                                ps[:, sl],
                                W7[kk][:, mt * 128:(mt + 1) * 128]
                                .bitcast(mm_dt),
                                net6[kk][:, sl].bitcast(mm_dt),
                                start=(kk == 0), stop=(kk == 3))
                    nc.scalar.activation(net7[mt][:, :], ps[:, :], AF.Relu,
                                         bias=B7[:, mt:mt + 1],
                                         scale=S7[:, mt:mt + 1])

                h8 = pp2.tile([P, HALF], F32)
                ps8 = hpsum.tile([P, HALF], F32, tag="hps")
                for j in range(2):
                    sl = slice(j * 512, (j + 1) * 512)
                    for kk in range(2):
                        nc.tensor.matmul(ps8[:, sl],
                                         W8[kk][:, :].bitcast(mm_dt),
                                         net7[kk][:, sl].bitcast(mm_dt),
                                         start=(kk == 0), stop=(kk == 1))
                nc.scalar.activation(h8[:, :], ps8[:, :], AF.Relu)
                lg = hpsmall.tile([1, HALF], F32, tag="hsm")
                for j in range(2):
                    sl = slice(j * 512, (j + 1) * 512)
                    nc.tensor.matmul(lg[:, sl], W9[:, :].bitcast(mm_dt),
                                     h8[:, sl].bitcast(mm_dt),
                                     start=True, stop=True)
                sc = pp2.tile([1, HALF], F32)
                nc.scalar.activation(sc[:, :], lg[:, :], AF.Sigmoid)
                nc.sync.dma_start(out=o_score[:, :], in_=sc[:, :])

                fsimT = pp2.tile([P, HALF], F32)
                psf = hpsum.tile([P, HALF], F32, tag="hps")
                for j in range(2):
                    sl = slice(j * 512, (j + 1) * 512)
                    for kk in range(2):
                        nc.tensor.matmul(psf[:, sl],
                                         W11[kk][:, :].bitcast(mm_dt),
                                         net7[kk][:, sl].bitcast(mm_dt),
                                         start=(kk == 0), stop=(kk == 1))
                nc.scalar.activation(fsimT[:, :], psf[:, :], AF.Relu)

                fsq = hb2.tile([P, HALF], F32, tag="fsq")
                nc.scalar.activation(fsq[:, :], fsimT[:, :], AF.Square)
                rrps = hpsmall.tile([1, HALF], F32, tag="hsm")
                for j in range(2):
                    nc.tensor.matmul(rrps[:, j * 512:(j + 1) * 512],
                                     ones_col[:, :],
                                     fsq[:, j * 512:(j + 1) * 512],
                                     start=True, stop=True)
                rr_own = pp2.tile([1, HALF], F32)
                nc.scalar.activation(rr_own[:, :], rrps[:, :], AF.Copy,
                                     bias=0.0, scale=1.0)

                ident = pp2.tile([P, P], F32)
                make_identity(nc, ident)
                for pt in range(8):
                    tp = hpsmall.tile([P, P], F32, tag="hsm")
                    nc.tensor.transpose(tp[:, :],
                                        fsimT[:, pt * 128:(pt + 1) * 128],
                                        ident[:, :])
                    fout = hb2.tile([P, P], F32, tag="fout")
                    nc.scalar.activation(fout[:, :], tp[:, :], AF.Copy,
                                         bias=0.0, scale=1.0)
                    nc.sync.dma_start(out=o_fsim[pt * 128:(pt + 1) * 128, :],
                                      in_=fout[:, :])

                rr_part = pp2.tile([P, 8], F32)
                for pt in range(8):
                    tp = hpsmall.tile([P, 1], F32, tag="hsm")
                    nc.tensor.transpose(tp[:, :],
                                        rr_own[:, pt * 128:(pt + 1) * 128],
                                        ident[0:1, 0:1])
                    nc.scalar.activation(rr_part[:, pt:pt + 1], tp[:, :],
                                         AF.Copy, bias=0.0, scale=1.0)

                nc.sync.dma_start(out=cc_in_fs[0:128, :], in_=fsimT[:, :])
                nc.sync.dma_start(out=cc_in_fs[128:129, :], in_=rr_own[:, :])
                nc.gpsimd.collective_compute(
                    "AllGather", ALU.bypass, replica_groups=GROUPS,
                    ins=[cc_in_fs.ap()], outs=[cc_out_fs.ap()])
                fsimTf = pp2.tile([P, N], F32)
                rr_full = pp2.tile([1, N], F32)
                for h in range(2):
                    nc.sync.dma_start(out=fsimTf[:, h * HALF:(h + 1) * HALF],
                                      in_=cc_out_fs[h, 0:128, :])
                    nc.sync.dma_start(out=rr_full[:, h * HALF:(h + 1) * HALF],
                                      in_=cc_out_fs[h, 128:129, :])
                m2f = pp2.tile([P, N], F32)
                nc.scalar.activation(m2f[:, :], fsimTf[:, :], AF.Copy,
                                     bias=0.0, scale=-2.0)

                for it in range(8):
                    sm = hb2.tile([P, N], F32, tag="sm")
                    for j in range(4):
                        sl = slice(j * 512, (j + 1) * 512)
                        ps = hpsmall.tile([P, 512], F32, tag="hsm")
                        nc.tensor.matmul(
                            ps[:, :],
                            fsimT[:, it * 128:(it + 1) * 128].bitcast(mm_dt),
                            m2f[:, sl].bitcast(mm_dt), start=True, stop=False)
                        nc.tensor.matmul(
                            ps[:, :], ones_row[:, :].bitcast(mm_dt),
                            rr_full[:, sl].bitcast(mm_dt),
                            start=False, stop=True)
                        nc.scalar.activation(sm[:, sl], ps[:, :], AF.Relu,
                                             bias=rr_part[:, it:it + 1],
                                             scale=1.0)
                    nc.sync.dma_start(out=o_simmat[it * 128:(it + 1) * 128, :],
                                      in_=sm[:, :])

    nc.finalize()
    return nc


_CACHE = {}


def _get_nc():
    if "nc" not in _CACHE:
        _CACHE["nc"] = build_nc()
    return _CACHE["nc"]


def _prep_maps(x, w1, bn1, w1_1, bn1_1, w2, bn2, w2_2, bn2_2, w3, bn3,
               w5, bn5, w6, bn6, w7, bn7, w8, w9, w11):
    def f(a):
        return np.ascontiguousarray(np.asarray(a, dtype=np.float32))

    r = 1.0 / np.sqrt(np.float32(1.0) + np.float32(EPS))

    def sb(bn, parts):
        s = (np.asarray(bn[0], np.float32) * r).astype(np.float32)
        b = np.asarray(bn[1], np.float32)
        if parts == 1:
            return f(s.reshape(-1, 1)), f(b.reshape(-1, 1))
        return f(s.reshape(parts, 128).T), f(b.reshape(parts, 128).T)

    s1v, b1v = sb(bn1, 1)
    s11v, b11v = sb(bn1_1, 1)
    s2v, b2v = sb(bn2, 1)
    s22v, b22v = sb(bn2_2, 1)
    s3v, b3v = sb(bn3, 1)
    s5v, b5v = sb(bn5, 8)
    s6v, b6v = sb(bn6, 4)
    s7v, b7v = sb(bn7, 2)

    w1 = f(w1); w2 = f(w2); w3 = f(w3)
    base = {
        "w1t": f(w1[0:6]), "w1b": f(w1[6:12] - w1[0:6]), "w1l2": f(w1_1),
        "s1": s1v, "b1": b1v, "s11": s11v, "b11": b11v,
        "w2t": f(w2[0:64]), "w2b": f(w2[64:128] - w2[0:64]), "w2l2": f(w2_2),
        "s2": s2v, "b2": b2v, "s22": s22v, "b22": b22v,
        "w3t": f(w3[0:64]), "w3b": f(w3[64:128] - w3[0:64]),
        "s3": s3v, "b3": b3v,
        "w5w": f(w5), "s5": s5v, "b5": b5v,
        "w6g": f(w6[0:1024]), "w6c": f(w6[1024:1216]),
        "s6": s6v, "b6": b6v,
        "w7w": f(w7), "s7": s7v, "b7": b7v,
        "w8w": f(w8), "w9w": f(w9), "w11sim": f(w11),
    }
    base["w11w"] = base.pop("w11sim")
    maps = []
    x = f(x)
    for c in range(8):
        b_, h_ = c // 2, c % 2
        m = dict(base)
        m["x_full"] = f(x[b_])
        m["x_own"] = f(x[b_][:, h_ * HALF:(h_ + 1) * HALF])
        maps.append(m)
    return maps


def _get_runner():
    """Cached jitted SPMD callable (mirrors bass2jax.run_bass_via_pjrt but
    keeps the compiled executable across kernel() calls)."""
    if "runner" in _CACHE:
        return _CACHE["runner"]
    import jax
    from jax.sharding import Mesh, PartitionSpec
    from jax.experimental.shard_map import shard_map
    from concourse import bass2jax
    from concourse import mybir as _mb

    nc = _get_nc()
    bass2jax.install_neuronx_cc_hook()
    partition_name = (nc.partition_id_tensor.name
                      if nc.partition_id_tensor else None)
    in_names, out_names, out_avals, zero_outs = [], [], [], []
    for alloc in nc.m.functions[0].allocations:
        if not isinstance(alloc, _mb.MemoryLocationSet):
            continue
        name = alloc.memorylocations[0].name
        if alloc.kind == "ExternalInput":
            if name != partition_name:
                in_names.append(name)
        elif alloc.kind == "ExternalOutput":
            out_names.append(name)
            shape = tuple(alloc.tensor_shape)
            dtype = _mb.dt.np(alloc.dtype)
            out_avals.append(jax.core.ShapedArray(shape, dtype))
            zero_outs.append(np.zeros(shape, dtype))
    n_params = len(in_names)
    n_outs = len(out_avals)
    all_in = list(in_names) + list(out_names)
    if partition_name is not None:
        all_in.append(partition_name)
    donate = tuple(range(n_params, n_params + n_outs))

    def _body(*args):
        operands = list(args)
        if partition_name is not None:
            operands.append(bass2jax.partition_id_tensor())
        outs = bass2jax._bass_exec_p.bind(
            *operands, out_avals=tuple(out_avals), in_names=tuple(all_in),
            out_names=tuple(out_names), lowering_input_output_aliases=(),
            sim_require_finite=True, sim_require_nnan=True, nc=nc)
        return tuple(outs)

    devices = jax.devices()[:8]
    mesh = Mesh(np.asarray(devices), ("core",))
    in_specs = (PartitionSpec("core"),) * (n_params + n_outs)
    out_specs = (PartitionSpec("core"),) * n_outs
    sharded = jax.jit(
        shard_map(_body, mesh=mesh, in_specs=in_specs, out_specs=out_specs,
                  check_rep=False),
        donate_argnums=donate, keep_unused=True)

    import jax.numpy as jnp
    from jax.sharding import NamedSharding
    zshard = tuple(NamedSharding(mesh, PartitionSpec("core"))
                   for _ in zero_outs)
    zfn = jax.jit(
        lambda: tuple(jnp.zeros((8 * z.shape[0], *z.shape[1:]), z.dtype)
                      for z in zero_outs),
        out_shardings=zshard)

    def run(maps):
        per_core = [[np.asarray(m[name]) for name in in_names[:n_params]]
                    for m in maps]
        concat_in = [np.concatenate([per_core[c][i] for c in range(8)], axis=0)
                     for i in range(n_params)]
        out_arrs = sharded(*concat_in, *zfn())
        return [
            {name: np.asarray(out_arrs[i]).reshape(8, *out_avals[i].shape)[c]
             for i, name in enumerate(out_names)}
            for c in range(8)
        ]

    _CACHE["runner"] = run
    return run


def kernel(**inputs):
    maps = _prep_maps(**inputs)
    rs = _get_runner()(maps)
    center = np.zeros((B, N), np.float32)
    fsim = np.zeros((B, N, 128), np.float32)
    simmat = np.zeros((B, N, N), np.float32)
    pdist = np.zeros((B, N, N), np.float32)
    for c in range(8):
        b_, h_ = c // 2, c % 2
        sl = slice(h_ * HALF, (h_ + 1) * HALF)
        center[b_, sl] = rs[c]["o_score"][0]
        fsim[b_, sl] = rs[c]["o_fsim"]
        simmat[b_, sl] = rs[c]["o_simmat"]
        pdist[b_, sl] = rs[c]["o_pdist"]
    return center, fsim, simmat, pdist

def _bench_device_impl(maps, reps=4):
    import time
    import jax
    from jax.sharding import Mesh, PartitionSpec
    from jax.experimental.shard_map import shard_map
    from concourse import bass2jax
    from concourse import mybir as _mb

    nc = _get_nc()
    bass2jax.install_neuronx_cc_hook()
    partition_name = (nc.partition_id_tensor.name
                      if nc.partition_id_tensor else None)
    in_names, out_names, out_avals, zero_outs = [], [], [], []
    for alloc in nc.m.functions[0].allocations:
        if not isinstance(alloc, _mb.MemoryLocationSet):
            continue
        name = alloc.memorylocations[0].name
        if alloc.kind == "ExternalInput":
            if name != partition_name:
                in_names.append(name)
        elif alloc.kind == "ExternalOutput":
            out_names.append(name)
            shape = tuple(alloc.tensor_shape)
            dtype = _mb.dt.np(alloc.dtype)
            out_avals.append(jax.core.ShapedArray(shape, dtype))
            zero_outs.append(np.zeros(shape, dtype))
    n_params = len(in_names)

    def _exec(ins, outs_prev):
        operands = list(ins) + list(outs_prev)
        if partition_name is not None:
            operands.append(bass2jax.partition_id_tensor())
        allnames = list(in_names) + list(out_names)
        if partition_name is not None:
            allnames.append(partition_name)
        return bass2jax._bass_exec_p.bind(
            *operands, out_avals=tuple(out_avals),
            in_names=tuple(allnames), out_names=tuple(out_names),
            lowering_input_output_aliases=(),
            sim_require_finite=True, sim_require_nnan=True, nc=nc)

    per_core = [[np.asarray(m[name]) for name in in_names[:n_params]]
                for m in maps]
    concat_in = [np.concatenate([per_core[c][i] for c in range(8)], axis=0)
                 for i in range(n_params)]
    concat_zeros = [np.zeros((8 * z.shape[0], *z.shape[1:]), z.dtype)
                    for z in zero_outs]
    devices = jax.devices()[:8]
    mesh = Mesh(np.asarray(devices), ("core",))

    def make(nrep):
        def body(*args):
            ins = args[:n_params]
            outs = list(args[n_params:])
            for _ in range(nrep):
                outs = list(_exec(ins, outs))
            return tuple(outs)
        return jax.jit(shard_map(
            body, mesh=mesh,
            in_specs=(PartitionSpec("core"),) * (n_params + len(zero_outs)),
            out_specs=(PartitionSpec("core"),) * len(zero_outs),
            check_rep=False), keep_unused=True)

    f1 = make(1)
    din = [jax.device_put(a) for a in concat_in]
    dzero = [jax.device_put(z) for z in concat_zeros]
    jax.block_until_ready(din); jax.block_until_ready(dzero)
    jax.block_until_ready(f1(*din, *dzero))
    ts = []
    for _ in range(reps):
        t0 = time.time()
        jax.block_until_ready(f1(*din, *dzero))
        ts.append(time.time() - t0)
    return min(ts), ts, None


# revision 25
# speedup vs baseline: 74.0840x; 1.0201x over previous
"""DGCNN-cls Trainium2 Bass kernel (8 NeuronCores, data-parallel batch+points).

Sharding: core c handles batch b=c//2, point rows [(c%2)*1024, (c%2)*1024+1024).
Pair collectives (AllGather features / AllReduce-max global pool) between cores
2b and 2b+1. kernel() accepts FULL inputs, returns FULL outputs.
"""

import numpy as np

import concourse.bass as bass
import concourse.bacc as bacc
import concourse.tile as tile
from concourse import mybir
from concourse.bass_utils import run_bass_kernel_spmd
from concourse.masks import make_identity

F32 = mybir.dt.float32
F32R = mybir.dt.float32r
U16 = mybir.dt.uint16
I16 = mybir.dt.int16

B = 4
C0 = 6
N = 2048
K = 20
HALF = 1024
NCHUNK = 8
P = 128
SEG = 64
NSEG = N // SEG
EPS = 1e-5
NEG = -1e30

AF = mybir.ActivationFunctionType
ALU = mybir.AluOpType


def _sap(ap, pairs, extra_off=0):
    """new AP on same tensor with custom [step,count] pairs."""
    return bass.AP(tensor=ap.tensor, offset=ap.offset + extra_off, ap=pairs)


def build_nc(mm_dt=F32, score_dt=F32):
    nc = bacc.Bacc("TRN2", target_bir_lowering=False, debug=False, num_devices=8)

    def inp(name, shape):
        return nc.dram_tensor(name, shape, F32, kind="ExternalInput")

    x_full = inp("x_full", [C0, N])
    x_own = inp("x_own", [C0, HALF])
    w1t = inp("w1t", [C0, 64]); w1b = inp("w1b", [C0, 64])
    w1l2 = inp("w1l2", [64, 64])
    s1 = inp("s1", [64, 1]); b1 = inp("b1", [64, 1])
    s11 = inp("s11", [64, 1]); b11 = inp("b11", [64, 1])
    w2t = inp("w2t", [64, 64]); w2b = inp("w2b", [64, 64])
    w2l2 = inp("w2l2", [64, 64])
    s2 = inp("s2", [64, 1]); b2 = inp("b2", [64, 1])
    s22 = inp("s22", [64, 1]); b22 = inp("b22", [64, 1])
    w3t = inp("w3t", [64, 64]); w3b = inp("w3b", [64, 64])
    s3 = inp("s3", [64, 1]); b3 = inp("b3", [64, 1])
    w5w = inp("w5w", [192, 1024])
    s5 = inp("s5", [P, 8]); b5 = inp("b5", [P, 8])
    w6g = inp("w6g", [1024, 512])
    w6c = inp("w6c", [192, 512])
    s6 = inp("s6", [P, 4]); b6 = inp("b6", [P, 4])
    w7w = inp("w7w", [512, 256])
    s7 = inp("s7", [P, 2]); b7 = inp("b7", [P, 2])
    w8w = inp("w8w", [256, 128])
    w9w = inp("w9w", [128, 1])
    w11w = inp("w11w", [256, 128])

    o_pdist = nc.dram_tensor("o_pdist", [HALF, N], F32, kind="ExternalOutput")
    o_simmat = nc.dram_tensor("o_simmat", [HALF, N], F32, kind="ExternalOutput")
    o_fsim = nc.dram_tensor("o_fsim", [HALF, 128], F32, kind="ExternalOutput")
    o_score = nc.dram_tensor("o_score", [1, HALF], F32, kind="ExternalOutput")

    idx_scr = nc.dram_tensor("idx_scr", [NCHUNK, P, 24], U16, kind="Internal")
    cc_in_x1 = nc.dram_tensor("cc_in_x1", [64, HALF], F32, kind="Internal")
    cc_out_x1 = nc.dram_tensor("cc_out_x1", [2, 64, HALF], F32, kind="Internal")
    cc_in_x2 = nc.dram_tensor("cc_in_x2", [64, HALF], F32, kind="Internal")
    cc_out_x2 = nc.dram_tensor("cc_out_x2", [2, 64, HALF], F32, kind="Internal")
    cc_in_g = nc.dram_tensor("cc_in_g", [P, 8], F32, kind="Internal")
    cc_out_g = nc.dram_tensor("cc_out_g", [P, 8], F32, kind="Internal")
    cc_in_fs = nc.dram_tensor("cc_in_fs", [129, HALF], F32, kind="Internal")
    cc_out_fs = nc.dram_tensor("cc_out_fs", [2, 129, HALF], F32, kind="Internal")
    GROUPS = [[0, 1], [2, 3], [4, 5], [6, 7]]

    with tile.TileContext(nc) as tc:
        with tc.tile_pool(name="persist", bufs=1) as pp:
            # ---------- weights ----------
            _ldn = [0]

            def load(t, rdt=None):
                _ldn[0] += 1
                tl = pp.tile(list(t.shape), F32, tag=f"ld{_ldn[0]}",
                             name=f"ld{_ldn[0]}")
                if rdt is None:
                    nc.sync.dma_start(out=tl[:, :], in_=t[:, :])
                else:
                    nc.sync.dma_start(out=tl[:, :].bitcast(rdt),
                                      in_=t[:, :].bitcast(rdt))
                return tl

            W1t, W1b, W1l2 = load(w1t), load(w1b), load(w1l2)
            S1, B1, S11, B11 = load(s1), load(b1), load(s11), load(b11)
            W2t, W2b, W2l2 = load(w2t), load(w2b), load(w2l2)
            S2, B2, S22, B22 = load(s2), load(b2), load(s22), load(b22)
            W3t, W3b = load(w3t), load(w3b)
            S3, B3 = load(s3), load(b3)
            W5a = pp.tile([128, 1024], F32)
            nc.sync.dma_start(out=W5a[:, :],
                              in_=w5w[0:128, :])
            W5b = pp.tile([64, 1024], F32)
            nc.sync.dma_start(out=W5b[:, :],
                              in_=w5w[128:192, :])
            S5, B5 = load(s5), load(b5)
            W6g = []
            for kk in range(8):
                t = pp.tile([128, 512], F32, tag=f"w6g{kk}", name=f"w6gt{kk}")
                nc.sync.dma_start(out=t[:, :], in_=w6g[kk * 128:(kk + 1) * 128, :])
                W6g.append(t)
            W6ca = pp.tile([128, 512], F32)
            nc.sync.dma_start(out=W6ca[:, :],
                              in_=w6c[0:128, :])
            W6cb = pp.tile([64, 512], F32)
            nc.sync.dma_start(out=W6cb[:, :],
                              in_=w6c[128:192, :])
            S6, B6 = load(s6), load(b6)
            W7 = []
            for kk in range(4):
                t = pp.tile([128, 256], F32, tag=f"w7_{kk}", name=f"w7t{kk}")
                nc.sync.dma_start(out=t[:, :], in_=w7w[kk * 128:(kk + 1) * 128, :])
                W7.append(t)
            S7, B7 = load(s7), load(b7)
            W8 = []
            for kk in range(2):
                t = pp.tile([128, 128], F32, tag=f"w8_{kk}", name=f"w8t{kk}")
                nc.sync.dma_start(out=t[:, :], in_=w8w[kk * 128:(kk + 1) * 128, :])
                W8.append(t)
            W9 = load(w9w)
            W11 = []
            for kk in range(2):
                t = pp.tile([128, 128], F32, tag=f"w11_{kk}", name=f"w11t{kk}")
                nc.sync.dma_start(out=t[:, :], in_=w11w[kk * 128:(kk + 1) * 128, :])
                W11.append(t)

            # ---------- features ----------
            f_ext = pp.tile([16, N], F32)
            nc.vector.memset(f_ext[:, :], 0.0)
            nc.sync.dma_start(out=f_ext[0:C0, :], in_=x_full[:, :])
            x_own_sb = pp.tile([C0, HALF], F32)
            nc.sync.dma_start(out=x_own_sb[:, :], in_=x_own[:, :])

            x1T = pp.tile([66, N], F32)
            x2T = pp.tile([66, N], F32)
            x1own = pp.tile([64, HALF], F32)
            x2own = pp.tile([64, HALF], F32)
            cat_a = pp.tile([P, HALF], F32)
            cat_b = pp.tile([64, HALF], F32)
            ones_col = pp.tile([P, 1], F32)
            nc.vector.memset(ones_col[:, :], 1.0)
            ones_row = pp.tile([1, P], F32)
            nc.vector.memset(ones_row[:, :], 1.0)
            ones_row_n = pp.tile([1, N], F32)
            nc.vector.memset(ones_row_n[:, :], 1.0)

            # ============ stages ============
            with (
                tc.tile_pool(name="stage", bufs=1) as sp,
                tc.tile_pool(name="sbuf2", bufs=2) as sp2,
                tc.tile_pool(name="spsum", bufs=1, space="PSUM") as spsum,
                tc.tile_pool(name="epsum", bufs=2, space="PSUM") as epsum,
            ):
                def colsq_row(dst_row_ap, src, nch, width, tagn):
                    """dst_row_ap [1,width] (any partition) = -sum_c src**2."""
                    sq = sp2.tile([64, width], F32, tag="sqtmp")
                    nc.scalar.activation(sq[0:nch, :], src, AF.Square)
                    ps = spsum.tile([1, width], F32, tag="spsum")
                    for j in range(0, width, 512):
                        nc.tensor.matmul(ps[:, j:j + 512], ones_col[0:nch, :],
                                         sq[0:nch, j:j + 512], start=True,
                                         stop=True)
                    row = sp.tile([1, width], F32, tag=tagn, name=tagn)
                    nc.scalar.activation(row[:, :], ps[:, :], AF.Copy,
                                         bias=0.0, scale=-1.0)
                    nc.sync.dma_start(out=dst_row_ap, in_=row[:, :])

                def run_stage(stage, feat_full, featC, qsrc, out_dst, out_p0):
                    CC = featC
                    KQ = CC + 2
                    if stage == 1:
                        Wt, Wb, Wl2 = W1t, W1b, W1l2
                        Sa, Ba, Sb, Bb = S1, B1, S11, B11
                    elif stage == 2:
                        Wt, Wb, Wl2 = W2t, W2b, W2l2
                        Sa, Ba, Sb, Bb = S2, B2, S22, B22
                    else:
                        Wt, Wb, Wl2 = W3t, W3b, None
                        Sa, Ba, Sb, Bb = S3, B3, None, None

                    colsq_row(feat_full[CC + 1:CC + 2, :], feat_full[0:CC, :],
                              CC, N, "nsqf")
                    nc.sync.dma_start(out=feat_full[CC:CC + 1, :],
                                      in_=ones_row_n[:, :])

                    q = sp.tile([66, HALF], F32, tag="qtile")
                    nc.scalar.activation(q[0:CC, :], qsrc, AF.Copy, bias=0.0,
                                         scale=2.0)
                    colsq_row(q[CC:CC + 1, :], qsrc, CC, HALF, "nsqq")
                    nc.sync.dma_start(out=q[CC + 1:CC + 2, :],
                                      in_=ones_row_n[:, 0:HALF])

                    idxw = sp.tile([64, NCHUNK * 160], I16, tag="idxw")

                    for m in range(NCHUNK):
                        ps = spsum.tile([P, N], F32, tag="spsum")
                        qs = q[0:KQ, m * P:(m + 1) * P]
                        for j in range(4):
                            nc.tensor.matmul(
                                ps[:, j * 512:(j + 1) * 512],
                                qs.bitcast(score_dt),
                                feat_full[0:KQ, j * 512:(j + 1) * 512]
                                .bitcast(score_dt),
                                start=True, stop=True)
                        s_sb = sp2.tile([P, N], F32, tag="s_sb")
                        nc.scalar.activation(s_sb[:, :], ps[:, :], AF.Copy,
                                             bias=0.0, scale=1.0)
                        if stage == 1:
                            d2 = sp2.tile([P, N], F32, tag="d2")
                            nc.scalar.activation(d2[:, :], ps[:, :], AF.Copy,
                                                 bias=0.0, scale=-1.0)
                            nc.sync.dma_start(
                                out=o_pdist[m * P:(m + 1) * P, :], in_=d2[:, :])

                        cands = sp.tile([P, 256], F32, tag="cands")
                        for g in range(NSEG):
                            nc.vector.max(cands[:, g * 8:(g + 1) * 8],
                                          s_sb[:, g * SEG:(g + 1) * SEG])
                        c1 = sp.tile([P, 256], F32, tag="c1")
                        c2 = sp.tile([P, 256], F32, tag="c2")
                        v8 = [sp.tile([P, 8], F32, tag=f"v8_{r}", name=f"v8t_{r}")
                              for r in range(3)]
                        nc.vector.max(v8[0][:, :], cands[:, :])
                        nc.vector.match_replace(c1[:, :], v8[0][:, :],
                                                cands[:, :], NEG)
                        nc.vector.max(v8[1][:, :], c1[:, :])
                        nc.vector.match_replace(c2[:, :], v8[1][:, :],
                                                c1[:, :], NEG)
                        nc.vector.max(v8[2][:, :], c2[:, :])
                        idx24 = sp.tile([P, 24], U16, tag="idx24")
                        for r in range(3):
                            nc.vector.max_index(idx24[:, r * 8:(r + 1) * 8],
                                                v8[r][:, :], s_sb[:, :])
                        nc.sync.dma_start(out=idx_scr[m, :, :], in_=idx24[:, :])

                    # readback wrapped idx: element j=k*128+p at [p%16, k*8+p//16]
                    ngrp = 4 if CC >= 16 else 1
                    for m in range(NCHUNK):
                        src = bass.AP(idx_scr, m * P * 24,
                                      [[24, 16], [16 * 24, 8], [1, 20]])
                        for g in range(ngrp):
                            nc.sync.dma_start(
                                out=idxw[g * 16:(g + 1) * 16,
                                         m * 160:(m + 1) * 160],
                                in_=src.bitcast(I16))

                    for m in range(NCHUNK):
                        nb = sp2.tile([64, K * P], F32, tag="nb")
                        if CC >= 16:
                            nc.gpsimd.ap_gather(
                                out_ap=nb[:, :], in_ap=feat_full[0:64, :],
                                idxs_ap=idxw[:, m * 160:(m + 1) * 160],
                                channels=64, num_elems=N, d=1, num_idxs=K * P)
                        else:
                            nc.gpsimd.ap_gather(
                                out_ap=nb[0:16, :], in_ap=f_ext[0:16, :],
                                idxs_ap=idxw[0:16, m * 160:(m + 1) * 160],
                                channels=16, num_elems=N, d=1, num_idxs=K * P)
                        h2 = sp2.tile([64, K * P], F32, tag="h2")
                        for pc in range(8):
                            sl = slice(pc * 320, (pc + 1) * 320)
                            c16 = qsrc[:, m * P + pc * 16:m * P + pc * 16 + 16]
                            ctr_b = _sap(c16, [c16.ap[0], [0, K], c16.ap[1]])
                            e1 = epsum.tile([64, 320], F32, tag="e1")
                            nc.tensor.matmul(e1[:, :], Wt[:, :].bitcast(mm_dt),
                                             nb[0:CC, sl].bitcast(mm_dt),
                                             start=True, stop=False)
                            nc.tensor.matmul(e1[:, :], Wb[:, :].bitcast(mm_dt),
                                             ctr_b.bitcast(mm_dt),
                                             start=False, stop=True)
                            if Wl2 is not None:
                                h1 = sp.tile([64, 320], F32, tag="h1")
                                nc.scalar.activation(h1[:, :],
                                                     e1[:, :],
                                                     AF.Relu, bias=Ba[:, :],
                                                     scale=Sa[:, :])
                                e2 = epsum.tile([64, 320], F32, tag="e2")
                                nc.tensor.matmul(e2[:, :],
                                                 Wl2[:, :],
                                                 h1[:, :],
                                                 start=True, stop=True)
                                nc.scalar.activation(h2[:, sl], e2[:, :],
                                                     AF.Relu, bias=Bb[:, :],
                                                     scale=Sb[:, :])
                            else:
                                nc.scalar.activation(h2[:, sl], e1[:, :],
                                                     AF.Relu, bias=Ba[:, :],
                                                     scale=Sa[:, :])
                        h2ap = h2[:, :]
                        red_in = _sap(h2ap, [h2ap.ap[0], [320, 8], [1, 16],
                                             [16, K]])
                        nc.vector.reduce_max(
                            out_dst[out_p0:out_p0 + 64, m * P:(m + 1) * P],
                            red_in, axis=mybir.AxisListType.X)

                run_stage(1, f_ext, C0, x_own_sb[:, :], x1own, 0)

                nc.sync.dma_start(out=cc_in_x1[:, :], in_=x1own[:, :])
                nc.sync.dma_start(out=cat_a[0:64, :], in_=x1own[:, :])
                nc.gpsimd.collective_compute(
                    "AllGather", ALU.bypass, replica_groups=GROUPS,
                    ins=[cc_in_x1.ap()], outs=[cc_out_x1.ap()])
                for h in range(2):
                    nc.sync.dma_start(out=x1T[0:64, h * HALF:(h + 1) * HALF],
                                      in_=cc_out_x1[h])

                run_stage(2, x1T, 64, x1own[:, :], x2own, 0)

                nc.sync.dma_start(out=cc_in_x2[:, :], in_=x2own[:, :])
                nc.sync.dma_start(out=cat_a[64:128, :], in_=x2own[:, :])
                nc.gpsimd.collective_compute(
                    "AllGather", ALU.bypass, replica_groups=GROUPS,
                    ins=[cc_in_x2.ap()], outs=[cc_out_x2.ap()])
                for h in range(2):
                    nc.sync.dma_start(out=x2T[0:64, h * HALF:(h + 1) * HALF],
                                      in_=cc_out_x2[h])

                run_stage(3, x2T, 64, x2own[:, :], cat_b, 0)

            # ============ head ============
            with (
                tc.tile_pool(name="persist2", bufs=1) as pp2,
                tc.tile_pool(name="h2buf", bufs=2) as hb2,
                tc.tile_pool(name="hpsum", bufs=2, space="PSUM") as hpsum,
                tc.tile_pool(name="hpsmall", bufs=2, space="PSUM") as hpsmall,
            ):
                pg = pp2.tile([P, 8], F32)
                for mt in range(8):
                    ps = hpsum.tile([P, HALF], F32, tag="hps")
                    for j in range(2):
                        sl = slice(j * 512, (j + 1) * 512)
                        nc.tensor.matmul(
                            ps[:, sl],
                            W5a[:, mt * 128:(mt + 1) * 128],
                            cat_a[:, sl], start=True, stop=False)
                        nc.tensor.matmul(
                            ps[:, sl],
                            W5b[:, mt * 128:(mt + 1) * 128],
                            cat_b[:, sl], start=False, stop=True)
                    h5 = hb2.tile([P, HALF], F32, tag="h5")
                    nc.scalar.activation(h5[:, :], ps[:, :], AF.Relu,
                                         bias=B5[:, mt:mt + 1],
                                         scale=S5[:, mt:mt + 1])
                    nc.vector.reduce_max(pg[:, mt:mt + 1], h5[:, :],
                                         axis=mybir.AxisListType.X)

                nc.sync.dma_start(out=cc_in_g[:, :], in_=pg[:, :])
                nc.gpsimd.collective_compute(
                    "AllReduce", ALU.max, replica_groups=GROUPS,
                    ins=[cc_in_g.ap()], outs=[cc_out_g.ap()])
                g_sb = pp2.tile([P, 8], F32)
                nc.sync.dma_start(out=g_sb[:, :], in_=cc_out_g[:, :])

                beta6 = pp2.tile([P, 4], F32)
                for mt in range(4):
                    ps = hpsmall.tile([P, 1], F32, tag="hsm")
                    for kk in range(8):
                        nc.tensor.matmul(
                            ps[:, :],
                            W6g[kk][:, mt * 128:(mt + 1) * 128],
                            g_sb[:, kk:kk + 1],
                            start=(kk == 0), stop=(kk == 7))
                    nc.vector.tensor_scalar(beta6[:, mt:mt + 1], ps[:, :],
                                            S6[:, mt:mt + 1], B6[:, mt:mt + 1],
                                            op0=ALU.mult, op1=ALU.add)

                net6 = [pp2.tile([P, HALF], F32, tag=f"net6_{i}", name=f"net6t_{i}")
                        for i in range(4)]
                for mt in range(4):
                    ps = hpsum.tile([P, HALF], F32, tag="hps")
                    for j in range(2):
                        sl = slice(j * 512, (j + 1) * 512)
                        nc.tensor.matmul(
                            ps[:, sl],
                            W6ca[:, mt * 128:(mt + 1) * 128],
                            cat_a[:, sl], start=True, stop=False)
                        nc.tensor.matmul(
                            ps[:, sl],
                            W6cb[:, mt * 128:(mt + 1) * 128],
                            cat_b[:, sl], start=False, stop=True)
                    nc.scalar.activation(net6[mt][:, :],
                                         ps[:, :], AF.Relu,
                                         bias=beta6[:, mt:mt + 1],
                                         scale=S6[:, mt:mt + 1])

                net7 = [pp2.tile([P, HALF], F32, tag=f"net7_{i}", name=f"net7t_{i}")
                        for i in range(2)]
                for mt in range(2):
                    ps = hpsum.tile([P, HALF], F32, tag="hps")
                    for j in range(2):
                        sl = slice(j * 512, (j + 1) * 512)
                        for kk in range(4):
                            nc.tensor.matmul(
                                ps[:, sl],
                                W7[kk][:, mt * 128:(mt + 1) * 128]
                                ,
                                net6[kk][:, sl],
                                start=(kk == 0), stop=(kk == 3))
                    nc.scalar.activation(net7[mt][:, :],
                                         ps[:, :], AF.Relu,
                                         bias=B7[:, mt:mt + 1],
                                         scale=S7[:, mt:mt + 1])

                h8 = pp2.tile([P, HALF], F32)
                ps8 = hpsum.tile([P, HALF], F32, tag="hps")
                for j in range(2):
                    sl = slice(j * 512, (j + 1) * 512)
                    for kk in range(2):
                        nc.tensor.matmul(ps8[:, sl],
                                         W8[kk][:, :],
                                         net7[kk][:, sl],
                                         start=(kk == 0), stop=(kk == 1))
                nc.scalar.activation(h8[:, :], ps8[:, :], AF.Relu)
                lg = hpsmall.tile([1, HALF], F32, tag="hsm")
                for j in range(2):
                    sl = slice(j * 512, (j + 1) * 512)
                    nc.tensor.matmul(lg[:, sl], W9[:, :],
                                     h8[:, sl],
                                     start=True, stop=True)
                sc = pp2.tile([1, HALF], F32)
                nc.scalar.activation(sc[:, :], lg[:, :], AF.Sigmoid)
                nc.sync.dma_start(out=o_score[:, :], in_=sc[:, :])

                fsimT = pp2.tile([P, HALF], F32)
                psf = hpsum.tile([P, HALF], F32, tag="hps")
                for j in range(2):
                    sl = slice(j * 512, (j + 1) * 512)
                    for kk in range(2):
                        nc.tensor.matmul(psf[:, sl],
                                         W11[kk][:, :],
                                         net7[kk][:, sl],
                                         start=(kk == 0), stop=(kk == 1))
                nc.scalar.activation(fsimT[:, :], psf[:, :], AF.Relu)

                fsq = hb2.tile([P, HALF], F32, tag="fsq")
                nc.scalar.activation(fsq[:, :], fsimT[:, :], AF.Square)
                rrps = hpsmall.tile([1, HALF], F32, tag="hsm")
                for j in range(2):
                    nc.tensor.matmul(rrps[:, j * 512:(j + 1) * 512],
                                     ones_col[:, :],
                                     fsq[:, j * 512:(j + 1) * 512],
                                     start=True, stop=True)
                rr_own = pp2.tile([1, HALF], F32)
                nc.scalar.activation(rr_own[:, :], rrps[:, :], AF.Copy,
                                     bias=0.0, scale=1.0)

                ident = pp2.tile([P, P], F32)
                make_identity(nc, ident)
                for pt in range(8):
                    tp = hpsmall.tile([P, P], F32, tag="hsm")
                    nc.tensor.transpose(tp[:, :],
                                        fsimT[:, pt * 128:(pt + 1) * 128],
                                        ident[:, :])
                    fout = hb2.tile([P, P], F32, tag="fout")
                    nc.scalar.activation(fout[:, :], tp[:, :], AF.Copy,
                                         bias=0.0, scale=1.0)
                    nc.sync.dma_start(out=o_fsim[pt * 128:(pt + 1) * 128, :],
                                      in_=fout[:, :])

                rr_part = pp2.tile([P, 8], F32)
                for pt in range(8):
                    tp = hpsmall.tile([P, 1], F32, tag="hsm")
                    nc.tensor.transpose(tp[:, :],
                                        rr_own[:, pt * 128:(pt + 1) * 128],
                                        ident[0:1, 0:1])
                    nc.scalar.activation(rr_part[:, pt:pt + 1], tp[:, :],
                                         AF.Copy, bias=0.0, scale=1.0)

                nc.sync.dma_start(out=cc_in_fs[0:128, :], in_=fsimT[:, :])
                nc.sync.dma_start(out=cc_in_fs[128:129, :], in_=rr_own[:, :])
                nc.gpsimd.collective_compute(
                    "AllGather", ALU.bypass, replica_groups=GROUPS,
                    ins=[cc_in_fs.ap()], outs=[cc_out_fs.ap()])
                fsimTf = pp2.tile([P, N], F32)
                rr_full = pp2.tile([1, N], F32)
                for h in range(2):
                    nc.sync.dma_start(
                        out=fsimTf[:, h * HALF:(h + 1) * HALF],
                        in_=cc_out_fs[h, 0:128, :])
                    nc.sync.dma_start(
                        out=rr_full[:, h * HALF:(h + 1) * HALF],
                        in_=cc_out_fs[h, 128:129, :])
                m2f = pp2.tile([P, N], F32)
                nc.scalar.activation(m2f[:, :], fsimTf[:, :],
                                     AF.Copy, bias=0.0, scale=-2.0)

                for it in range(8):
                    sm = hb2.tile([P, N], F32, tag="sm")
                    for j in range(4):
                        sl = slice(j * 512, (j + 1) * 512)
                        ps = hpsmall.tile([P, 512], F32, tag="hsm")
                        nc.tensor.matmul(
                            ps[:, :],
                            fsimT[:, it * 128:(it + 1) * 128],
                            m2f[:, sl], start=True, stop=False)
                        nc.tensor.matmul(
                            ps[:, :], ones_row[:, :], rr_full[:, sl],
                            start=False, stop=True)
                        nc.scalar.activation(sm[:, sl], ps[:, :], AF.Relu,
                                             bias=rr_part[:, it:it + 1],
                                             scale=1.0)
                    nc.sync.dma_start(out=o_simmat[it * 128:(it + 1) * 128, :],
                                      in_=sm[:, :])

    nc.finalize()
    return nc


_CACHE = {}


def _get_nc():
    if "nc" not in _CACHE:
        _CACHE["nc"] = build_nc()
    return _CACHE["nc"]


def _prep_maps(x, w1, bn1, w1_1, bn1_1, w2, bn2, w2_2, bn2_2, w3, bn3,
               w5, bn5, w6, bn6, w7, bn7, w8, w9, w11):
    def f(a):
        return np.ascontiguousarray(np.asarray(a, dtype=np.float32))

    r = 1.0 / np.sqrt(np.float32(1.0) + np.float32(EPS))

    def sb(bn, parts):
        s = (np.asarray(bn[0], np.float32) * r).astype(np.float32)
        b = np.asarray(bn[1], np.float32)
        if parts == 1:
            return f(s.reshape(-1, 1)), f(b.reshape(-1, 1))
        return f(s.reshape(parts, 128).T), f(b.reshape(parts, 128).T)

    s1v, b1v = sb(bn1, 1)
    s11v, b11v = sb(bn1_1, 1)
    s2v, b2v = sb(bn2, 1)
    s22v, b22v = sb(bn2_2, 1)
    s3v, b3v = sb(bn3, 1)
    s5v, b5v = sb(bn5, 8)
    s6v, b6v = sb(bn6, 4)
    s7v, b7v = sb(bn7, 2)

    w1 = f(w1); w2 = f(w2); w3 = f(w3)
    base = {
        "w1t": f(w1[0:6]), "w1b": f(w1[6:12] - w1[0:6]), "w1l2": f(w1_1),
        "s1": s1v, "b1": b1v, "s11": s11v, "b11": b11v,
        "w2t": f(w2[0:64]), "w2b": f(w2[64:128] - w2[0:64]), "w2l2": f(w2_2),
        "s2": s2v, "b2": b2v, "s22": s22v, "b22": b22v,
        "w3t": f(w3[0:64]), "w3b": f(w3[64:128] - w3[0:64]),
        "s3": s3v, "b3": b3v,
        "w5w": f(w5), "s5": s5v, "b5": b5v,
        "w6g": f(w6[0:1024]), "w6c": f(w6[1024:1216]),
        "s6": s6v, "b6": b6v,
        "w7w": f(w7), "s7": s7v, "b7": b7v,
        "w8w": f(w8), "w9w": f(w9), "w11sim": f(w11),
    }
    base["w11w"] = base.pop("w11sim")
    maps = []
    x = f(x)
    for c in range(8):
        b_, h_ = c // 2, c % 2
        m = dict(base)
        m["x_full"] = f(x[b_])
        m["x_own"] = f(x[b_][:, h_ * HALF:(h_ + 1) * HALF])
        maps.append(m)
    return maps


def _get_runner():
    """Cached jitted SPMD callable (mirrors bass2jax.run_bass_via_pjrt but
    keeps the compiled executable across kernel() calls)."""
    if "runner" in _CACHE:
        return _CACHE["runner"]
    import jax
    from jax.sharding import Mesh, PartitionSpec
    from jax.experimental.shard_map import shard_map
    from concourse import bass2jax
    from concourse import mybir as _mb

    nc = _get_nc()
    bass2jax.install_neuronx_cc_hook()
    partition_name = (nc.partition_id_tensor.name
                      if nc.partition_id_tensor else None)
    in_names, out_names, out_avals, zero_outs = [], [], [], []
    for alloc in nc.m.functions[0].allocations:
        if not isinstance(alloc, _mb.MemoryLocationSet):
            continue
        name = alloc.memorylocations[0].name
        if alloc.kind == "ExternalInput":
            if name != partition_name:
                in_names.append(name)
        elif alloc.kind == "ExternalOutput":
            out_names.append(name)
            shape = tuple(alloc.tensor_shape)
            dtype = _mb.dt.np(alloc.dtype)
            out_avals.append(jax.core.ShapedArray(shape, dtype))
            zero_outs.append(np.zeros(shape, dtype))
    n_params = len(in_names)
    n_outs = len(out_avals)
    all_in = list(in_names) + list(out_names)
    if partition_name is not None:
        all_in.append(partition_name)
    donate = tuple(range(n_params, n_params + n_outs))

    def _body(*args):
        operands = list(args)
        if partition_name is not None:
            operands.append(bass2jax.partition_id_tensor())
        outs = bass2jax._bass_exec_p.bind(
            *operands, out_avals=tuple(out_avals), in_names=tuple(all_in),
            out_names=tuple(out_names), lowering_input_output_aliases=(),
            sim_require_finite=True, sim_require_nnan=True, nc=nc)
        return tuple(outs)

    devices = jax.devices()[:8]
    mesh = Mesh(np.asarray(devices), ("core",))
    in_specs = (PartitionSpec("core"),) * (n_params + n_outs)
    out_specs = (PartitionSpec("core"),) * n_outs
    sharded = jax.jit(
        shard_map(_body, mesh=mesh, in_specs=in_specs, out_specs=out_specs,
                  check_rep=False),
        donate_argnums=donate, keep_unused=True)

    import jax.numpy as jnp
    from jax.sharding import NamedSharding
    zshard = tuple(NamedSharding(mesh, PartitionSpec("core"))
                   for _ in zero_outs)
    zfn = jax.jit(
        lambda: tuple(jnp.zeros((8 * z.shape[0], *z.shape[1:]), z.dtype)
                      for z in zero_outs),
        out_shardings=zshard)

    def run(maps):
        per_core = [[np.asarray(m[name]) for name in in_names[:n_params]]
                    for m in maps]
        concat_in = [np.concatenate([per_core[c][i] for c in range(8)], axis=0)
                     for i in range(n_params)]
        out_arrs = sharded(*concat_in, *zfn())
        return [
            {name: np.asarray(out_arrs[i]).reshape(8, *out_avals[i].shape)[c]
             for i, name in enumerate(out_names)}
            for c in range(8)
        ]

    _CACHE["runner"] = run
    return run


def kernel(**inputs):
    maps = _prep_maps(**inputs)
    rs = _get_runner()(maps)
    center = np.zeros((B, N), np.float32)
    fsim = np.zeros((B, N, 128), np.float32)
    simmat = np.zeros((B, N, N), np.float32)
    pdist = np.zeros((B, N, N), np.float32)
    for c in range(8):
        b_, h_ = c // 2, c % 2
        sl = slice(h_ * HALF, (h_ + 1) * HALF)
        center[b_, sl] = rs[c]["o_score"][0]
        fsim[b_, sl] = rs[c]["o_fsim"]
        simmat[b_, sl] = rs[c]["o_simmat"]
        pdist[b_, sl] = rs[c]["o_pdist"]
    return center, fsim, simmat, pdist

def _bench_device_impl(maps, reps=4):
    import time
    import jax
    from jax.sharding import Mesh, PartitionSpec
    from jax.experimental.shard_map import shard_map
    from concourse import bass2jax
    from concourse import mybir as _mb

    nc = _get_nc()
    bass2jax.install_neuronx_cc_hook()
    partition_name = (nc.partition_id_tensor.name
                      if nc.partition_id_tensor else None)
    in_names, out_names, out_avals, zero_outs = [], [], [], []
    for alloc in nc.m.functions[0].allocations:
        if not isinstance(alloc, _mb.MemoryLocationSet):
            continue
        name = alloc.memorylocations[0].name
        if alloc.kind == "ExternalInput":
            if name != partition_name:
                in_names.append(name)
        elif alloc.kind == "ExternalOutput":
            out_names.append(name)
            shape = tuple(alloc.tensor_shape)
            dtype = _mb.dt.np(alloc.dtype)
            out_avals.append(jax.core.ShapedArray(shape, dtype))
            zero_outs.append(np.zeros(shape, dtype))
    n_params = len(in_names)

    def _exec(ins, outs_prev):
        operands = list(ins) + list(outs_prev)
        if partition_name is not None:
            operands.append(bass2jax.partition_id_tensor())
        allnames = list(in_names) + list(out_names)
        if partition_name is not None:
            allnames.append(partition_name)
        return bass2jax._bass_exec_p.bind(
            *operands, out_avals=tuple(out_avals),
            in_names=tuple(allnames), out_names=tuple(out_names),
            lowering_input_output_aliases=(),
            sim_require_finite=True, sim_require_nnan=True, nc=nc)

    per_core = [[np.asarray(m[name]) for name in in_names[:n_params]]
                for m in maps]
    concat_in = [np.concatenate([per_core[c][i] for c in range(8)], axis=0)
                 for i in range(n_params)]
    concat_zeros = [np.zeros((8 * z.shape[0], *z.shape[1:]), z.dtype)
                    for z in zero_outs]
    devices = jax.devices()[:8]
    mesh = Mesh(np.asarray(devices), ("core",))

    def make(nrep):
        def body(*args):
            ins = args[:n_params]
            outs = list(args[n_params:])
            for _ in range(nrep):
                outs = list(_exec(ins, outs))
            return tuple(outs)
        return jax.jit(shard_map(
            body, mesh=mesh,
            in_specs=(PartitionSpec("core"),) * (n_params + len(zero_outs)),
            out_specs=(PartitionSpec("core"),) * len(zero_outs),
            check_rep=False), keep_unused=True)

    f1 = make(1)
    din = [jax.device_put(a) for a in concat_in]
    dzero = [jax.device_put(z) for z in concat_zeros]
    jax.block_until_ready(din); jax.block_until_ready(dzero)
    jax.block_until_ready(f1(*din, *dzero))
    ts = []
    for _ in range(reps):
        t0 = time.time()
        jax.block_until_ready(f1(*din, *dzero))
        ts.append(time.time() - t0)
    return min(ts), ts, None
